# revision 1
# baseline (speedup 1.0000x reference)
"""Trainium2 Bass kernel for a GAT-style GNN layer (8 NeuronCores, SPMD).

Reference computation:
    h = x @ W                                  # [N, FOUT]
    e = leakyrelu(Wh1[row] + Wh2[col])         # per-edge scores
    att = softmax(e, axis=1)                   # axis of size 1 -> exactly 1.0
    out = elu(segment_sum(att * h[col], row))  # [N, FOUT]

Because the softmax is over a size-1 axis, att == 1.0 exactly, so the kernel
computes out = elu(segment_sum(h[col], row)) and `a` is unused.

Strategy (2 SPMD launches over 8 cores, sharded by destination-node range):
  Launch 1: h = x @ W for this core's node slice, stored as an interleaved
            dual-fp16 table h2[n] = [fp16(h[n]), fp16(h[n]-hi)] (~2^-22 rel).
  Host:     replicate h2 to all cores; bucket+sort edges by
            (dest window of 128 nodes, source chunk of 25000 nodes), pad each
            (window, chunk) group to a multiple of 128 and to the max over
            cores so all cores share one static program.
  Launch 2: dma_gather messages from h2; per 128-edge block build a one-hot
            S = (iota == row_local) and accumulate S.T @ msg into PSUM
            (segment sum); ELU; write the node-range slice of the output.
"""

import os
from contextlib import ExitStack
from dataclasses import dataclass, field

import ml_dtypes
import numpy as np

import concourse.bass as bass
import concourse.tile as tile
from concourse import bacc, library_config, mybir
from concourse.bass_utils import run_bass_kernel_spmd

F32 = mybir.dt.float32
F16 = mybir.dt.float16
BF16 = mybir.dt.bfloat16
I16 = mybir.dt.int16

P = 128


@dataclass(frozen=True)
class Config:
    n: int = 100000          # nodes
    fin: int = 256           # input features
    fout: int = 128          # output features
    ncores: int = 8
    nchunk: int = 4          # gather-table chunks (int16 index limit)
    gwin: int = 3            # windows per gather super-group
    dual: bool = True        # dual-fp16 (hi+lo) message table vs single fp16
    trim: bool = False       # trailing -1 padding (not a win: instr overhead)

    @property
    def npc(self):
        return self.n // self.ncores

    @property
    def nwin(self):
        return (self.npc + P - 1) // P

    @property
    def chunk(self):
        return self.n // self.nchunk

    @property
    def twidth(self):
        return (2 if self.dual else 1) * self.fout


CFG = Config()


def _split_hi_lo(x, dt):
    hi = x.astype(dt)
    lo = (x - hi.astype(np.float32)).astype(dt)
    return hi, lo


# --------------------------------------------------------------------------
# Static edge layout (shared across cores -> one SPMD program)
# --------------------------------------------------------------------------

@dataclass
class Layout:
    cfg: Config
    swc: np.ndarray          # [nwin*nchunk] padded group sizes (mult of 128)
    off: np.ndarray          # [nwin*nchunk] slot offset of each group
    runs: list               # [(g, c, start, length)] gather runs
    groups: list             # list of window-index lists
    total_slots: int = 0
    m16: np.ndarray = None   # static per-(w,c) valid count incl. zero-fill
    notrim: np.ndarray = None  # first-rotation groups: fully gathered

    @property
    def nblocks(self):
        return self.total_slots // P


def build_layout(cfg: Config, row, col):
    """row/col: int arrays [E]. Returns (layout, per-core stream builders)."""
    npc, nwin, nchunk, chunk = cfg.npc, cfg.nwin, cfg.nchunk, cfg.chunk
    ngrp = nwin * nchunk

    per_core = []
    counts = np.zeros((cfg.ncores, ngrp), np.int64)
    for k in range(cfg.ncores):
        sel = (row >= k * npc) & (row < (k + 1) * npc)
        r = (row[sel] - k * npc).astype(np.int64)
        c_ = col[sel].astype(np.int64)
        w = r // P
        rl = r - w * P
        ch = c_ // chunk
        cl = c_ - ch * chunk
        key = w * nchunk + ch
        counts[k] = np.bincount(key, minlength=ngrp)
        per_core.append((key, cl, rl))

    cmax = counts.max(axis=0)
    swc = ((cmax + P - 1) // P) * P               # 128-padded static sizes
    m16 = ((cmax + 15) // 16) * 16                # static valid count (reg)

    groups = [list(range(g, min(g + cfg.gwin, nwin)))
              for g in range(0, nwin, cfg.gwin)]

    off = np.zeros(ngrp, np.int64)
    runs = []
    cur = 0
    for gi, g in enumerate(groups):
        for c in range(nchunk):
            start = cur
            for w in g:
                off[w * nchunk + c] = cur
                cur += swc[w * nchunk + c]
            runs.append((gi, c, start, cur - start))
    layout = Layout(cfg, swc, off, runs, groups, int(cur))
    layout.m16 = m16
    notrim = np.zeros(ngrp, bool)
    for g in groups[:4]:
        for w in g:
            notrim[w * nchunk:(w + 1) * nchunk] = True
    layout.notrim = notrim
    return layout, per_core


def build_streams(layout: Layout, key, cl, rl):
    """Per-core edge streams: wrapped int16 gather indices + row-local values."""
    cfg = layout.cfg
    total = layout.total_slots
    pad = -1 if cfg.trim else 0
    idx_local = np.full(total, pad, np.int16)
    rowloc = np.full(total, -1.0, np.float32)

    order = np.argsort(key, kind="stable")
    skey = key[order]
    scl = cl[order]
    srl = rl[order]
    cnt = np.bincount(key, minlength=len(layout.swc))
    starts = np.concatenate([[0], np.cumsum(cnt)[:-1]])
    rank = np.arange(len(skey)) - starts[skey]
    slot = layout.off[skey] + rank
    idx_local[slot] = scl.astype(np.int16)
    rowloc[slot] = srl

    if cfg.trim:
        for gidx in range(len(layout.swc)):
            o = int(layout.off[gidx])
            if layout.notrim[gidx]:
                b = int(layout.swc[gidx])
            else:
                b = int(layout.m16[gidx])
            a = int(cnt[gidx])
            if b > a:
                idx_local[o + a:o + b] = 0
    idx_w = np.zeros((P, total // 16), np.int16)
    for (_, _, s, L) in layout.runs:
        if L == 0:
            continue
        seg = idx_local[s:s + L].reshape(-1, 16).T          # [16, L/16]
        idx_w[:, s // 16:(s + L) // 16] = np.tile(seg, (8, 1))
    rl_w = np.ascontiguousarray(
        rowloc.reshape(-1, P).T.astype(np.float16))          # [128, nblocks]
    return idx_w, rl_w


# --------------------------------------------------------------------------
# Launch 1: h = x @ W  (node-sharded), emit interleaved fp16 table
# --------------------------------------------------------------------------

def build_phase1(cfg: Config):
    nc = bacc.Bacc("TRN2", target_bir_lowering=False, debug=False,
                   num_devices=cfg.ncores)
    fin, fout, npc = cfg.fin, cfg.fout, cfg.npc
    nkt = fin // P
    xt_hi = nc.dram_tensor("xt_hi", [fin, npc], BF16, kind="ExternalInput")
    xt_lo = nc.dram_tensor("xt_lo", [fin, npc], BF16, kind="ExternalInput")
    w_hi = nc.dram_tensor("w_hi", [fin, fout], BF16, kind="ExternalInput")
    w_lo = nc.dram_tensor("w_lo", [fin, fout], BF16, kind="ExternalInput")
    h2 = nc.dram_tensor("h2", [npc, cfg.twidth], F16, kind="ExternalOutput")

    with tile.TileContext(nc) as tc, ExitStack() as ctx:
        wpool = ctx.enter_context(tc.tile_pool(name="w", bufs=1))
        xpool = ctx.enter_context(tc.tile_pool(name="x", bufs=1))
        ppool = ctx.enter_context(tc.tile_pool(name="ps", bufs=4, space="PSUM"))
        opool = ctx.enter_context(tc.tile_pool(name="o", bufs=4))

        whs, wls = [], []
        for k in range(nkt):
            wh = wpool.tile([P, fout], BF16, tag=f"wh{k}")
            wl = wpool.tile([P, fout], BF16, tag=f"wl{k}")
            nc.sync.dma_start(wh[:], w_hi.ap()[k * P:(k + 1) * P, :])
            nc.sync.dma_start(wl[:], w_lo.ap()[k * P:(k + 1) * P, :])
            whs.append(wh)
            wls.append(wl)

        # Whole x-transpose slice resident in SBUF: 4 big line-rate DMAs
        # instead of 4 small ones per node tile (HWDGE issue overhead).
        xh, xl = [], []
        for k in range(nkt):
            a = xpool.tile([P, npc], BF16, tag=f"xh{k}")
            b = xpool.tile([P, npc], BF16, tag=f"xl{k}")
            nc.sync.dma_start(a[:], xt_hi.ap()[k * P:(k + 1) * P, :])
            nc.sync.dma_start(b[:], xt_lo.ap()[k * P:(k + 1) * P, :])
            xh.append(a)
            xl.append(b)

        for t in range(cfg.nwin):
            n0 = t * P
            nt = min(npc - n0, P)
            ps = ppool.tile([P, fout], F32)
            mms = []
            for k in range(nkt):
                mms += [(xh[k], whs[k]), (xh[k], wls[k]), (xl[k], whs[k])]
            for j, (lhsT, rhs) in enumerate(mms):
                nc.tensor.matmul(ps[:nt, :], lhsT[:, n0:n0 + nt], rhs[:],
                                 start=(j == 0), stop=(j == len(mms) - 1))
            ot = opool.tile([P, cfg.twidth], F16, tag="ot")
            nc.vector.tensor_copy(ot[:nt, 0:fout], ps[:nt, :])
            if cfg.dual:
                t32 = opool.tile([P, fout], F32, tag="t32")
                nc.vector.tensor_copy(t32[:nt, :], ot[:nt, 0:fout])
                nc.vector.tensor_tensor(ot[:nt, fout:2 * fout], ps[:nt, :],
                                        t32[:nt, :], op=mybir.AluOpType.subtract)
            nc.sync.dma_start(h2.ap()[n0:n0 + nt, :], ot[:nt, :])
    nc.compile()
    return nc


# --------------------------------------------------------------------------
# Launch 2: gather + segment-sum (one-hot matmul) + ELU
# --------------------------------------------------------------------------

def build_phase2(cfg: Config, layout: Layout):
    nc = bacc.Bacc("TRN2", target_bir_lowering=False, debug=False,
                   num_devices=cfg.ncores, num_swdge_queues=4)
    fout, npc, nchunk, chunk = cfg.fout, cfg.npc, cfg.nchunk, cfg.chunk
    tw = cfg.twidth
    h2 = nc.dram_tensor("h2", [cfg.n, tw], F16, kind="ExternalInput")
    idxs = nc.dram_tensor("idxs", [P, layout.total_slots // 16], I16,
                          kind="ExternalInput")
    rowloc = nc.dram_tensor("rowloc", [P, layout.nblocks], F16,
                            kind="ExternalInput")
    SBATCH = 8  # one-hot builds batched over consecutive blocks
    iota_in = nc.dram_tensor("iota", [P, SBATCH * P], F16, kind="ExternalInput")
    out = nc.dram_tensor("out", [npc, fout], F32, kind="ExternalOutput")

    run_by_gc = {(g, c): (s, L) for (g, c, s, L) in layout.runs}
    bmax = {c: max((run_by_gc[(gi, c)][1] // P)
                   for gi in range(len(layout.groups))) for c in range(nchunk)}

    with tile.TileContext(nc) as tc, ExitStack() as ctx:
        nc.gpsimd.load_library(library_config.mlp)

        cpool = ctx.enter_context(tc.tile_pool(name="const", bufs=1))
        MBUFS = 4
        mpool = ctx.enter_context(tc.tile_pool(name="msg", bufs=MBUFS))
        spool = ctx.enter_context(tc.tile_pool(name="sel", bufs=6))
        ppool = ctx.enter_context(tc.tile_pool(name="ps", bufs=4, space="PSUM"))
        epool = ctx.enter_context(tc.tile_pool(name="elu", bufs=3))

        iota_t = cpool.tile([P, SBATCH * P], F16)
        nc.sync.dma_start(iota_t[:], iota_in.ap()[:, :])
        rl_t = cpool.tile([P, layout.nblocks], F16)
        nc.sync.dma_start(rl_t[:], rowloc.ap()[:, :])
        # whole idx stream resident: removes per-run DMAs + waits from the
        # gather issue path
        idx_t = cpool.tile([P, layout.total_slots // 16], I16)
        nc.sync.dma_start(idx_t[:], idxs.ap()[:, :])

        for gi, g in enumerate(layout.groups):
            mts = {}
            for c in range(nchunk):
                s, L = run_by_gc[(gi, c)]
                if L == 0:
                    continue
                mt = mpool.tile([P, bmax[c], tw], F16, tag=f"msg{c}")
                # single_packet=True (fast CounterMachine path) caps one
                # gather at 64 descs/engine = 1024 indices. With trim, one
                # gather per (window, chunk) subgroup so that -1 padding is
                # trailing within each instruction (the Q7 trims it).
                if cfg.trim:
                    bounds = []
                    for w in g:
                        sw = int(layout.swc[w * nchunk + c])
                        o = int(layout.off[w * nchunk + c])
                        mv = int(layout.m16[w * nchunk + c])
                        nsub = (sw + 1023) // 1024
                        full = bool(layout.notrim[w * nchunk + c])
                        for j in range(nsub):
                            sub = -(-(sw // P) // nsub) * P
                            a = o + j * sub
                            b = min(o + sw, a + sub)
                            if b > a:
                                rv = (b - a if full else
                                      min(max(mv - (a - o), 0), b - a))
                                bounds.append((a, b, rv))
                else:
                    nsub = (L + 1023) // 1024
                    sub = -(-(L // P) // nsub) * P
                    bounds = [(o, min(o + sub, s + L), min(o + sub, s + L) - o)
                              for o in range(s, s + L, sub)]
                for (a, b, rv) in bounds:
                    Lg = b - a
                    nc.gpsimd.dma_gather(
                        mt[:, (a - s) // P:(b - s) // P, :],
                        h2.ap()[c * chunk:(c + 1) * chunk, :],
                        idx_t[:, a // 16:b // 16], Lg, rv, tw,
                        single_packet=True, queue_num=c % 4)
                mts[c] = mt
            for w in g:
                blocks = []
                for c in range(nchunk):
                    o = layout.off[w * nchunk + c]
                    s, _ = run_by_gc[(gi, c)]
                    nb = layout.swc[w * nchunk + c] // P
                    for j in range(nb):
                        blocks.append((c, (o - s) // P + j, o // P + j))
                nt = min(npc - w * P, P)
                ot = epool.tile([P, fout], F32, tag="out")
                if not blocks:
                    nc.vector.memset(ot[:], 0.0)
                    nc.sync.dma_start(out.ap()[w * P:w * P + nt, :], ot[:nt, :])
                    continue
                ps = ppool.tile([P, fout], F32)
                nmm = len(blocks) * (2 if cfg.dual else 1)
                mi = 0
                # batch one-hot builds over runs of consecutive global blocks
                bi = 0
                while bi < len(blocks):
                    nb = 1
                    while (nb < SBATCH and bi + nb < len(blocks)
                           and blocks[bi + nb][2] == blocks[bi][2] + nb):
                        nb += 1
                    gb0 = blocks[bi][2]
                    sel = spool.tile([P, SBATCH * P], F16, tag="sel")
                    nc.vector.tensor_tensor(
                        sel[:, :nb * P], iota_t[:, :nb * P],
                        rl_t[:, gb0:gb0 + nb].to_broadcast([P, nb, P]),
                        op=mybir.AluOpType.is_equal)
                    for j in range(nb):
                        c, lb, _ = blocks[bi + j]
                        st = sel[:, j * P:(j + 1) * P]
                        nc.tensor.matmul(ps[:], st, mts[c][:, lb, 0:fout],
                                         start=(mi == 0), stop=(mi == nmm - 1))
                        mi += 1
                        if cfg.dual:
                            nc.tensor.matmul(ps[:], st,
                                             mts[c][:, lb, fout:2 * fout],
                                             start=False, stop=(mi == nmm - 1))
                            mi += 1
                    bi += nb
                # ELU: relu(x) - 1 + exp(min(x, 0))
                tmin = epool.tile([P, fout], F32, tag="tmin")
                texp = epool.tile([P, fout], F32, tag="texp")
                trel = epool.tile([P, fout], F32, tag="trel")
                nc.scalar.activation(tmin[:], ps[:],
                                     mybir.ActivationFunctionType.Relu,
                                     scale=-1.0)
                nc.scalar.activation(texp[:], tmin[:],
                                     mybir.ActivationFunctionType.Exp,
                                     scale=-1.0)
                nc.vector.tensor_scalar(trel[:], ps[:], 0.0, -1.0,
                                        mybir.AluOpType.max,
                                        mybir.AluOpType.add)
                nc.vector.tensor_add(ot[:], texp[:], trel[:])
                nc.sync.dma_start(out.ap()[w * P:w * P + nt, :], ot[:nt, :])
    nc.compile()
    return nc


# --------------------------------------------------------------------------
# Host orchestration
# --------------------------------------------------------------------------

_P1_CACHE = {}
_P2_CACHE = {}


def _phase1_nc(cfg: Config):
    key = (cfg.n, cfg.fin, cfg.fout, cfg.ncores, cfg.dual)
    if key not in _P1_CACHE:
        _P1_CACHE[key] = build_phase1(cfg)
    return _P1_CACHE[key]


def _phase2_nc(cfg: Config, layout: Layout):
    key = (cfg.n, cfg.fin, cfg.fout, cfg.ncores, cfg.dual, cfg.gwin,
           cfg.trim, tuple(layout.swc.tolist()))
    if key not in _P2_CACHE:
        _P2_CACHE[key] = build_phase2(cfg, layout)
    return _P2_CACHE[key]


def run(x, edge_index, W, a=None, cfg: Config = CFG, trace=False):
    """Full pipeline; returns (out, info dict with exec times)."""
    x = np.asarray(x, np.float32)
    W = np.asarray(W, np.float32)
    edge_index = np.asarray(edge_index)
    row = edge_index[0].astype(np.int64)
    col = edge_index[1].astype(np.int64)
    npc = cfg.npc
    info = {}

    # ---- phase 1 ----
    x_hi, x_lo = _split_hi_lo(x, ml_dtypes.bfloat16)
    w_hi, w_lo = _split_hi_lo(W, ml_dtypes.bfloat16)
    xt_hi = np.ascontiguousarray(x_hi.T)
    xt_lo = np.ascontiguousarray(x_lo.T)
    nc1 = _phase1_nc(cfg)
    in1 = [{
        "xt_hi": np.ascontiguousarray(xt_hi[:, k * npc:(k + 1) * npc]),
        "xt_lo": np.ascontiguousarray(xt_lo[:, k * npc:(k + 1) * npc]),
        "w_hi": w_hi, "w_lo": w_lo,
    } for k in range(cfg.ncores)]
    r1 = run_bass_kernel_spmd(nc1, in1, list(range(cfg.ncores)), trace=trace)
    h2 = np.concatenate([r1.results[k]["h2"] for k in range(cfg.ncores)], axis=0)
    info["p1_ns"] = r1.exec_time_ns

    # ---- layout + streams ----
    layout, per_core = build_layout(cfg, row, col)
    nc2 = _phase2_nc(cfg, layout)
    iota = np.ascontiguousarray(
        np.broadcast_to(np.tile(np.arange(P, dtype=np.float16), 8), (P, 8 * P)))
    in2 = []
    for k in range(cfg.ncores):
        idx_w, rl_w = build_streams(layout, *per_core[k])
        in2.append({"h2": h2, "idxs": idx_w, "rowloc": rl_w, "iota": iota})
    r2 = run_bass_kernel_spmd(nc2, in2, list(range(cfg.ncores)), trace=trace)
    out = np.concatenate([r2.results[k]["out"] for k in range(cfg.ncores)],
                         axis=0)
    info["p2_ns"] = r2.exec_time_ns
    info["total_slots"] = layout.total_slots
    info["results"] = (r1, r2)
    return out, info


def kernel(x, edge_index, W, a=None, **_ignored):
    out, _ = run(x, edge_index, W, a)
    return out



# revision 11
# speedup vs baseline: 1.0890x; 1.0890x over previous
"""Trainium2 Bass kernel for a GAT-style GNN layer (8 NeuronCores, SPMD).

Reference computation:
    h = x @ W                                  # [N, FOUT]
    e = leakyrelu(Wh1[row] + Wh2[col])         # per-edge scores
    att = softmax(e, axis=1)                   # axis of size 1 -> exactly 1.0
    out = elu(segment_sum(att * h[col], row))  # [N, FOUT]

Because the softmax is over a size-1 axis, att == 1.0 exactly, so the kernel
computes out = elu(segment_sum(h[col], row)) and `a` is unused.

Strategy (2 SPMD launches over 8 cores, sharded by destination-node range):
  Launch 1: h = x @ W for this core's node slice, stored as an interleaved
            dual-fp16 table h2[n] = [fp16(h[n]), fp16(h[n]-hi)] (~2^-22 rel).
  Host:     replicate h2 to all cores; bucket+sort edges by
            (dest window of 128 nodes, source chunk of 25000 nodes), pad each
            (window, chunk) group to a multiple of 128 and to the max over
            cores so all cores share one static program.
  Launch 2: dma_gather messages from h2; per 128-edge block build a one-hot
            S = (iota == row_local) and accumulate S.T @ msg into PSUM
            (segment sum); ELU; write the node-range slice of the output.
"""

import os
from contextlib import ExitStack
from dataclasses import dataclass, field

import ml_dtypes
import numpy as np

import concourse.bass as bass
import concourse.tile as tile
from concourse import bacc, library_config, mybir
from concourse.bass_utils import run_bass_kernel_spmd

F32 = mybir.dt.float32
F16 = mybir.dt.float16
BF16 = mybir.dt.bfloat16
I16 = mybir.dt.int16

P = 128


@dataclass(frozen=True)
class Config:
    n: int = 100000          # nodes
    fin: int = 256           # input features
    fout: int = 128          # output features
    ncores: int = 8
    nchunk: int = 4          # gather-table chunks (int16 index limit)
    gwin: int = 3            # windows per gather super-group
    dual: bool = True        # dual-fp16 (hi+lo) message table vs single fp16
    trim: bool = False       # trailing -1 padding (not a win: instr overhead)
    gmax: int = 1024         # max indices per gather instr (>1024 => multi-packet)
    mbufs: int = 4           # message pool buffers
    sptr: bool = False       # one-hot via per-block tensor_scalar (4x DVE)
    p1single: bool = False   # phase1: x in single bf16, W dual; chunked loads

    @property
    def npc(self):
        return self.n // self.ncores

    @property
    def nwin(self):
        return (self.npc + P - 1) // P

    @property
    def chunk(self):
        return self.n // self.nchunk

    @property
    def twidth(self):
        return (2 if self.dual else 1) * self.fout


CFG = Config()


def _split_hi_lo(x, dt):
    hi = x.astype(dt)
    lo = (x - hi.astype(np.float32)).astype(dt)
    return hi, lo


# --------------------------------------------------------------------------
# Static edge layout (shared across cores -> one SPMD program)
# --------------------------------------------------------------------------

@dataclass
class Layout:
    cfg: Config
    swc: np.ndarray          # [nwin*nchunk] padded group sizes (mult of 128)
    off: np.ndarray          # [nwin*nchunk] slot offset of each group
    runs: list               # [(g, c, start, length)] gather runs
    groups: list             # list of window-index lists
    total_slots: int = 0
    m16: np.ndarray = None   # static per-(w,c) valid count incl. zero-fill
    notrim: np.ndarray = None  # first-rotation groups: fully gathered

    @property
    def nblocks(self):
        return self.total_slots // P


def build_layout(cfg: Config, row, col):
    """row/col: int arrays [E]. Returns (layout, per-core stream builders)."""
    npc, nwin, nchunk, chunk = cfg.npc, cfg.nwin, cfg.nchunk, cfg.chunk
    ngrp = nwin * nchunk

    per_core = []
    counts = np.zeros((cfg.ncores, ngrp), np.int64)
    for k in range(cfg.ncores):
        sel = (row >= k * npc) & (row < (k + 1) * npc)
        r = (row[sel] - k * npc).astype(np.int64)
        c_ = col[sel].astype(np.int64)
        w = r // P
        rl = r - w * P
        ch = c_ // chunk
        cl = c_ - ch * chunk
        key = w * nchunk + ch
        counts[k] = np.bincount(key, minlength=ngrp)
        per_core.append((key, cl, rl))

    cmax = counts.max(axis=0)
    swc = ((cmax + P - 1) // P) * P               # 128-padded static sizes
    m16 = ((cmax + 15) // 16) * 16                # static valid count (reg)

    groups = [list(range(g, min(g + cfg.gwin, nwin)))
              for g in range(0, nwin, cfg.gwin)]

    off = np.zeros(ngrp, np.int64)
    runs = []
    cur = 0
    for gi, g in enumerate(groups):
        for c in range(nchunk):
            start = cur
            for w in g:
                off[w * nchunk + c] = cur
                cur += swc[w * nchunk + c]
            runs.append((gi, c, start, cur - start))
    layout = Layout(cfg, swc, off, runs, groups, int(cur))
    layout.m16 = m16
    notrim = np.zeros(ngrp, bool)
    for g in groups[:4]:
        for w in g:
            notrim[w * nchunk:(w + 1) * nchunk] = True
    layout.notrim = notrim
    return layout, per_core


def build_streams(layout: Layout, key, cl, rl):
    """Per-core edge streams: wrapped int16 gather indices + row-local values."""
    cfg = layout.cfg
    total = layout.total_slots
    pad = -1 if cfg.trim else 0
    idx_local = np.full(total, pad, np.int16)
    rowloc = np.full(total, -1.0, np.float32)

    order = np.argsort(key, kind="stable")
    skey = key[order]
    scl = cl[order]
    srl = rl[order]
    cnt = np.bincount(key, minlength=len(layout.swc))
    starts = np.concatenate([[0], np.cumsum(cnt)[:-1]])
    rank = np.arange(len(skey)) - starts[skey]
    slot = layout.off[skey] + rank
    idx_local[slot] = scl.astype(np.int16)
    rowloc[slot] = srl

    if cfg.trim:
        for gidx in range(len(layout.swc)):
            o = int(layout.off[gidx])
            if layout.notrim[gidx]:
                b = int(layout.swc[gidx])
            else:
                b = int(layout.m16[gidx])
            a = int(cnt[gidx])
            if b > a:
                idx_local[o + a:o + b] = 0
    idx_w = np.zeros((P, total // 16), np.int16)
    for (_, _, s, L) in layout.runs:
        if L == 0:
            continue
        seg = idx_local[s:s + L].reshape(-1, 16).T          # [16, L/16]
        idx_w[:, s // 16:(s + L) // 16] = np.tile(seg, (8, 1))
    rl_w = np.ascontiguousarray(
        rowloc.reshape(-1, P).T.astype(np.float16))          # [128, nblocks]
    return idx_w, rl_w


# --------------------------------------------------------------------------
# Launch 1: h = x @ W  (node-sharded), emit interleaved fp16 table
# --------------------------------------------------------------------------

def build_phase1(cfg: Config):
    nc = bacc.Bacc("TRN2", target_bir_lowering=False, debug=False,
                   num_devices=cfg.ncores)
    fin, fout, npc = cfg.fin, cfg.fout, cfg.npc
    nkt = fin // P
    xt_hi = nc.dram_tensor("xt_hi", [fin, npc], BF16, kind="ExternalInput")
    xt_lo = nc.dram_tensor("xt_lo", [fin, npc], BF16, kind="ExternalInput")
    w_hi = nc.dram_tensor("w_hi", [fin, fout], BF16, kind="ExternalInput")
    w_lo = nc.dram_tensor("w_lo", [fin, fout], BF16, kind="ExternalInput")
    h2 = nc.dram_tensor("h2", [npc, cfg.twidth], F16, kind="ExternalOutput")

    with tile.TileContext(nc) as tc, ExitStack() as ctx:
        wpool = ctx.enter_context(tc.tile_pool(name="w", bufs=1))
        xpool = ctx.enter_context(tc.tile_pool(name="x", bufs=1))
        ppool = ctx.enter_context(tc.tile_pool(name="ps", bufs=4, space="PSUM"))
        opool = ctx.enter_context(tc.tile_pool(name="o", bufs=4))

        whs, wls = [], []
        for k in range(nkt):
            wh = wpool.tile([P, fout], BF16, tag=f"wh{k}")
            wl = wpool.tile([P, fout], BF16, tag=f"wl{k}")
            nc.sync.dma_start(wh[:], w_hi.ap()[k * P:(k + 1) * P, :])
            nc.sync.dma_start(wl[:], w_lo.ap()[k * P:(k + 1) * P, :])
            whs.append(wh)
            wls.append(wl)

        # Whole x-transpose slice resident in SBUF: 4 big line-rate DMAs
        # instead of 4 small ones per node tile (HWDGE issue overhead).
        xh, xl = [], []
        for k in range(nkt):
            a = xpool.tile([P, npc], BF16, tag=f"xh{k}")
            b = xpool.tile([P, npc], BF16, tag=f"xl{k}")
            nc.sync.dma_start(a[:], xt_hi.ap()[k * P:(k + 1) * P, :])
            nc.sync.dma_start(b[:], xt_lo.ap()[k * P:(k + 1) * P, :])
            xh.append(a)
            xl.append(b)

        for t in range(cfg.nwin):
            n0 = t * P
            nt = min(npc - n0, P)
            ps = ppool.tile([P, fout], F32)
            mms = []
            for k in range(nkt):
                mms += [(xh[k], whs[k]), (xh[k], wls[k]), (xl[k], whs[k])]
            for j, (lhsT, rhs) in enumerate(mms):
                nc.tensor.matmul(ps[:nt, :], lhsT[:, n0:n0 + nt], rhs[:],
                                 start=(j == 0), stop=(j == len(mms) - 1))
            ot = opool.tile([P, cfg.twidth], F16, tag="ot")
            nc.vector.tensor_copy(ot[:nt, 0:fout], ps[:nt, :])
            if cfg.dual:
                t32 = opool.tile([P, fout], F32, tag="t32")
                nc.vector.tensor_copy(t32[:nt, :], ot[:nt, 0:fout])
                nc.vector.tensor_tensor(ot[:nt, fout:2 * fout], ps[:nt, :],
                                        t32[:nt, :], op=mybir.AluOpType.subtract)
            nc.sync.dma_start(h2.ap()[n0:n0 + nt, :], ot[:nt, :])
    nc.compile()
    return nc


def build_phase1_single(cfg: Config):
    """h = x @ W with x in single bf16 (W dual-bf16): 2 matmuls per k-tile.

    x loads are chunked along the node axis so the first window's matmuls
    start after ~1/8 of the x slice has landed instead of all of it.
    """
    nc = bacc.Bacc("TRN2", target_bir_lowering=False, debug=False,
                   num_devices=cfg.ncores)
    fin, fout, npc = cfg.fin, cfg.fout, cfg.npc
    nkt = fin // P
    xt = nc.dram_tensor("xt", [fin, npc], BF16, kind="ExternalInput")
    w_hi = nc.dram_tensor("w_hi", [fin, fout], BF16, kind="ExternalInput")
    w_lo = nc.dram_tensor("w_lo", [fin, fout], BF16, kind="ExternalInput")
    h2 = nc.dram_tensor("h2", [npc, cfg.twidth], F16, kind="ExternalOutput")

    NXC = 8  # x column chunks
    xcb = [(npc * j // NXC // P * P) for j in range(NXC)] + [npc]

    with tile.TileContext(nc) as tc, ExitStack() as ctx:
        wpool = ctx.enter_context(tc.tile_pool(name="w", bufs=1))
        xpool = ctx.enter_context(tc.tile_pool(name="x", bufs=1))
        ppool = ctx.enter_context(tc.tile_pool(name="ps", bufs=4, space="PSUM"))
        opool = ctx.enter_context(tc.tile_pool(name="o", bufs=4))

        whs, wls = [], []
        for k in range(nkt):
            wh = wpool.tile([P, fout], BF16, tag=f"wh{k}")
            wl = wpool.tile([P, fout], BF16, tag=f"wl{k}")
            nc.sync.dma_start(wh[:], w_hi.ap()[k * P:(k + 1) * P, :])
            nc.sync.dma_start(wl[:], w_lo.ap()[k * P:(k + 1) * P, :])
            whs.append(wh)
            wls.append(wl)

        xh = [[None] * NXC for _ in range(nkt)]
        for j in range(NXC):
            c0, c1 = xcb[j], xcb[j + 1]
            for k in range(nkt):
                a = xpool.tile([P, c1 - c0], BF16, tag=f"xh{k}_{j}")
                nc.sync.dma_start(a[:], xt.ap()[k * P:(k + 1) * P, c0:c1])
                xh[k][j] = a

        for t in range(cfg.nwin):
            n0 = t * P
            nt = min(npc - n0, P)
            j = next(i for i in range(NXC) if xcb[i] <= n0 < xcb[i + 1])
            o0 = n0 - xcb[j]
            ps = ppool.tile([P, fout], F32)
            mms = []
            for k in range(nkt):
                mms += [(xh[k][j], whs[k]), (xh[k][j], wls[k])]
            for i, (lhsT, rhs) in enumerate(mms):
                nc.tensor.matmul(ps[:nt, :], lhsT[:, o0:o0 + nt], rhs[:],
                                 start=(i == 0), stop=(i == len(mms) - 1))
            ot = opool.tile([P, cfg.twidth], F16, tag="ot")
            nc.vector.tensor_copy(ot[:nt, 0:fout], ps[:nt, :])
            if cfg.dual:
                t32 = opool.tile([P, fout], F32, tag="t32")
                nc.vector.tensor_copy(t32[:nt, :], ot[:nt, 0:fout])
                nc.vector.tensor_tensor(ot[:nt, fout:2 * fout], ps[:nt, :],
                                        t32[:nt, :], op=mybir.AluOpType.subtract)
            nc.sync.dma_start(h2.ap()[n0:n0 + nt, :], ot[:nt, :])
    nc.compile()
    return nc


# --------------------------------------------------------------------------
# Launch 2: gather + segment-sum (one-hot matmul) + ELU
# --------------------------------------------------------------------------

def build_phase2(cfg: Config, layout: Layout):
    nc = bacc.Bacc("TRN2", target_bir_lowering=False, debug=False,
                   num_devices=cfg.ncores, num_swdge_queues=4)
    fout, npc, nchunk, chunk = cfg.fout, cfg.npc, cfg.nchunk, cfg.chunk
    tw = cfg.twidth
    h2 = nc.dram_tensor("h2", [cfg.n, tw], F16, kind="ExternalInput")
    idxs = nc.dram_tensor("idxs", [P, layout.total_slots // 16], I16,
                          kind="ExternalInput")
    rowloc = nc.dram_tensor("rowloc", [P, layout.nblocks], F16,
                            kind="ExternalInput")
    SBATCH = 8  # one-hot builds batched over consecutive blocks
    iota_in = nc.dram_tensor("iota", [P, SBATCH * P], F16, kind="ExternalInput")
    out = nc.dram_tensor("out", [npc, fout], F32, kind="ExternalOutput")

    run_by_gc = {(g, c): (s, L) for (g, c, s, L) in layout.runs}
    bmax = {c: max((run_by_gc[(gi, c)][1] // P)
                   for gi in range(len(layout.groups))) for c in range(nchunk)}

    with tile.TileContext(nc) as tc, ExitStack() as ctx:
        nc.gpsimd.load_library(library_config.mlp)

        cpool = ctx.enter_context(tc.tile_pool(name="const", bufs=1))
        MBUFS = cfg.mbufs
        mpool = ctx.enter_context(tc.tile_pool(name="msg", bufs=MBUFS))
        spool = ctx.enter_context(tc.tile_pool(name="sel", bufs=6))
        ppool = ctx.enter_context(tc.tile_pool(name="ps", bufs=4, space="PSUM"))
        epool = ctx.enter_context(tc.tile_pool(name="elu", bufs=3))

        iota_t = cpool.tile([P, SBATCH * P], F16)
        nc.sync.dma_start(iota_t[:], iota_in.ap()[:, :])
        rl_t = cpool.tile([P, layout.nblocks], F16)
        nc.sync.dma_start(rl_t[:], rowloc.ap()[:, :])
        # whole idx stream resident: removes per-run DMAs + waits from the
        # gather issue path
        idx_t = cpool.tile([P, layout.total_slots // 16], I16)
        nc.sync.dma_start(idx_t[:], idxs.ap()[:, :])

        for gi, g in enumerate(layout.groups):
            mts = {}
            for c in range(nchunk):
                s, L = run_by_gc[(gi, c)]
                if L == 0:
                    continue
                mt = mpool.tile([P, bmax[c], tw], F16, tag=f"msg{c}")
                # single_packet=True (fast CounterMachine path) caps one
                # gather at 64 descs/engine = 1024 indices. With trim, one
                # gather per (window, chunk) subgroup so that -1 padding is
                # trailing within each instruction (the Q7 trims it).
                if cfg.trim:
                    bounds = []
                    for w in g:
                        sw = int(layout.swc[w * nchunk + c])
                        o = int(layout.off[w * nchunk + c])
                        mv = int(layout.m16[w * nchunk + c])
                        nsub = (sw + 1023) // 1024
                        full = bool(layout.notrim[w * nchunk + c])
                        for j in range(nsub):
                            sub = -(-(sw // P) // nsub) * P
                            a = o + j * sub
                            b = min(o + sw, a + sub)
                            if b > a:
                                rv = (b - a if full else
                                      min(max(mv - (a - o), 0), b - a))
                                bounds.append((a, b, rv))
                else:
                    nsub = (L + cfg.gmax - 1) // cfg.gmax
                    sub = -(-(L // P) // nsub) * P
                    bounds = [(o, min(o + sub, s + L), min(o + sub, s + L) - o)
                              for o in range(s, s + L, sub)]
                for (a, b, rv) in bounds:
                    Lg = b - a
                    nc.gpsimd.dma_gather(
                        mt[:, (a - s) // P:(b - s) // P, :],
                        h2.ap()[c * chunk:(c + 1) * chunk, :],
                        idx_t[:, a // 16:b // 16], Lg, rv, tw,
                        single_packet=(Lg <= 1024), queue_num=c % 4)
                mts[c] = mt
            for w in g:
                blocks = []
                for c in range(nchunk):
                    o = layout.off[w * nchunk + c]
                    s, _ = run_by_gc[(gi, c)]
                    nb = layout.swc[w * nchunk + c] // P
                    for j in range(nb):
                        blocks.append((c, (o - s) // P + j, o // P + j))
                nt = min(npc - w * P, P)
                ot = epool.tile([P, fout], F32, tag="out")
                if not blocks:
                    nc.vector.memset(ot[:], 0.0)
                    nc.sync.dma_start(out.ap()[w * P:w * P + nt, :], ot[:nt, :])
                    continue
                ps = ppool.tile([P, fout], F32)
                nmm = len(blocks) * (2 if cfg.dual else 1)
                mi = 0
                if cfg.sptr:
                    # per-block one-hot via tensor_scalar with per-partition
                    # scalar (InstTensorScalarPtr -> 4x DVE mode)
                    for (c, lb, gb) in blocks:
                        st = spool.tile([P, P], F16, tag="sel")
                        nc.vector.tensor_scalar(
                            st[:], iota_t[:, :P], rl_t[:, gb:gb + 1], None,
                            mybir.AluOpType.is_equal)
                        nc.tensor.matmul(ps[:], st[:], mts[c][:, lb, 0:fout],
                                         start=(mi == 0), stop=(mi == nmm - 1))
                        mi += 1
                        if cfg.dual:
                            nc.tensor.matmul(ps[:], st[:],
                                             mts[c][:, lb, fout:2 * fout],
                                             start=False, stop=(mi == nmm - 1))
                            mi += 1
                    blocks = []
                # batch one-hot builds over runs of consecutive global blocks
                bi = 0
                while bi < len(blocks):
                    nb = 1
                    while (nb < SBATCH and bi + nb < len(blocks)
                           and blocks[bi + nb][2] == blocks[bi][2] + nb):
                        nb += 1
                    gb0 = blocks[bi][2]
                    sel = spool.tile([P, SBATCH * P], F16, tag="sel")
                    nc.vector.tensor_tensor(
                        sel[:, :nb * P], iota_t[:, :nb * P],
                        rl_t[:, gb0:gb0 + nb].to_broadcast([P, nb, P]),
                        op=mybir.AluOpType.is_equal)
                    for j in range(nb):
                        c, lb, _ = blocks[bi + j]
                        st = sel[:, j * P:(j + 1) * P]
                        nc.tensor.matmul(ps[:], st, mts[c][:, lb, 0:fout],
                                         start=(mi == 0), stop=(mi == nmm - 1))
                        mi += 1
                        if cfg.dual:
                            nc.tensor.matmul(ps[:], st,
                                             mts[c][:, lb, fout:2 * fout],
                                             start=False, stop=(mi == nmm - 1))
                            mi += 1
                    bi += nb
                # ELU: relu(x) - 1 + exp(min(x, 0))
                tmin = epool.tile([P, fout], F32, tag="tmin")
                texp = epool.tile([P, fout], F32, tag="texp")
                trel = epool.tile([P, fout], F32, tag="trel")
                nc.scalar.activation(tmin[:], ps[:],
                                     mybir.ActivationFunctionType.Relu,
                                     scale=-1.0)
                nc.scalar.activation(texp[:], tmin[:],
                                     mybir.ActivationFunctionType.Exp,
                                     scale=-1.0)
                nc.vector.tensor_scalar(trel[:], ps[:], 0.0, -1.0,
                                        mybir.AluOpType.max,
                                        mybir.AluOpType.add)
                nc.vector.tensor_add(ot[:], texp[:], trel[:])
                nc.sync.dma_start(out.ap()[w * P:w * P + nt, :], ot[:nt, :])
    nc.compile()
    return nc


# --------------------------------------------------------------------------
# Host orchestration
# --------------------------------------------------------------------------

_P1_CACHE = {}
_P2_CACHE = {}


def _phase1_nc(cfg: Config):
    key = (cfg.n, cfg.fin, cfg.fout, cfg.ncores, cfg.dual, cfg.p1single)
    if key not in _P1_CACHE:
        _P1_CACHE[key] = (build_phase1_single(cfg) if cfg.p1single
                          else build_phase1(cfg))
    return _P1_CACHE[key]


def _phase2_nc(cfg: Config, layout: Layout):
    key = (cfg.n, cfg.fin, cfg.fout, cfg.ncores, cfg.dual, cfg.gwin,
           cfg.trim, cfg.gmax, cfg.mbufs, cfg.sptr,
           tuple(layout.swc.tolist()))
    if key not in _P2_CACHE:
        _P2_CACHE[key] = build_phase2(cfg, layout)
    return _P2_CACHE[key]


def run(x, edge_index, W, a=None, cfg: Config = CFG, trace=False):
    """Full pipeline; returns (out, info dict with exec times)."""
    x = np.asarray(x, np.float32)
    W = np.asarray(W, np.float32)
    edge_index = np.asarray(edge_index)
    row = edge_index[0].astype(np.int64)
    col = edge_index[1].astype(np.int64)
    npc = cfg.npc
    info = {}

    # ---- phase 1 ----
    x_hi, x_lo = _split_hi_lo(x, ml_dtypes.bfloat16)
    w_hi, w_lo = _split_hi_lo(W, ml_dtypes.bfloat16)
    xt_hi = np.ascontiguousarray(x_hi.T)
    nc1 = _phase1_nc(cfg)
    if cfg.p1single:
        in1 = [{
            "xt": np.ascontiguousarray(xt_hi[:, k * npc:(k + 1) * npc]),
            "w_hi": w_hi, "w_lo": w_lo,
        } for k in range(cfg.ncores)]
    else:
        xt_lo = np.ascontiguousarray(x_lo.T)
        in1 = [{
            "xt_hi": np.ascontiguousarray(xt_hi[:, k * npc:(k + 1) * npc]),
            "xt_lo": np.ascontiguousarray(xt_lo[:, k * npc:(k + 1) * npc]),
            "w_hi": w_hi, "w_lo": w_lo,
        } for k in range(cfg.ncores)]
    r1 = run_bass_kernel_spmd(nc1, in1, list(range(cfg.ncores)), trace=trace)
    h2 = np.concatenate([r1.results[k]["h2"] for k in range(cfg.ncores)], axis=0)
    info["p1_ns"] = r1.exec_time_ns

    # ---- layout + streams ----
    layout, per_core = build_layout(cfg, row, col)
    nc2 = _phase2_nc(cfg, layout)
    iota = np.ascontiguousarray(
        np.broadcast_to(np.tile(np.arange(P, dtype=np.float16), 8), (P, 8 * P)))
    in2 = []
    for k in range(cfg.ncores):
        idx_w, rl_w = build_streams(layout, *per_core[k])
        in2.append({"h2": h2, "idxs": idx_w, "rowloc": rl_w, "iota": iota})
    r2 = run_bass_kernel_spmd(nc2, in2, list(range(cfg.ncores)), trace=trace)
    out = np.concatenate([r2.results[k]["out"] for k in range(cfg.ncores)],
                         axis=0)
    info["p2_ns"] = r2.exec_time_ns
    info["total_slots"] = layout.total_slots
    info["results"] = (r1, r2)
    return out, info


def kernel(x, edge_index, W, a=None, **_ignored):
    out, _ = run(x, edge_index, W, a)
    return out



# revision 44
# speedup vs baseline: 1.3593x; 1.2482x over previous
"""Trainium2 Bass kernel for a GAT-style GNN layer (8 NeuronCores, SPMD).

Reference computation:
    h = x @ W                                  # [N, FOUT]
    e = leakyrelu(Wh1[row] + Wh2[col])         # per-edge scores
    att = softmax(e, axis=1)                   # axis of size 1 -> exactly 1.0
    out = elu(segment_sum(att * h[col], row))  # [N, FOUT]

Because the softmax is over a size-1 axis, att == 1.0 exactly, so the kernel
computes out = elu(segment_sum(h[col], row)) and `a` is unused.

Strategy (2 SPMD launches over 8 cores, sharded by destination-node range):
  Launch 1: h = x @ W for this core's node slice, stored as an interleaved
            dual-fp16 table h2[n] = [fp16(h[n]), fp16(h[n]-hi)] (~2^-22 rel).
  Host:     replicate h2 to all cores; bucket+sort edges by
            (dest window of 128 nodes, source chunk of 25000 nodes), pad each
            (window, chunk) group to a multiple of 128 and to the max over
            cores so all cores share one static program.
  Launch 2: dma_gather messages from h2; per 128-edge block build a one-hot
            S = (iota == row_local) and accumulate S.T @ msg into PSUM
            (segment sum); ELU; write the node-range slice of the output.
"""

import os
from contextlib import ExitStack
from dataclasses import dataclass, field

import ml_dtypes
import numpy as np

import concourse.bass as bass
import concourse.tile as tile
from concourse import bacc, library_config, mybir
from concourse.bass_utils import run_bass_kernel_spmd

F32 = mybir.dt.float32
F16 = mybir.dt.float16
BF16 = mybir.dt.bfloat16
I16 = mybir.dt.int16

P = 128


@dataclass(frozen=True)
class Config:
    n: int = 100000          # nodes
    fin: int = 256           # input features
    fout: int = 128          # output features
    ncores: int = 8
    nchunk: int = 4          # gather-table chunks (int16 index limit)
    gwin: int = 3            # windows per gather super-group
    dual: bool = False       # dual-fp16 (hi+lo) message table vs single fp16
    trim: bool = False       # trailing -1 padding (not a win: instr overhead)
    gmax: int = 1024         # max indices per gather instr (>1024 => multi-packet)
    mbufs: int = 4           # message pool buffers
    sptr: bool = False       # one-hot via per-block tensor_scalar (4x DVE)
    p1single: bool = False   # phase1: x in single bf16, W dual; chunked loads
    compact: bool = True     # 16-granular buckets + straddle blocks
    p1t: bool = True         # phase1: transposed h output (wide tiles)
    paired: bool = True      # 512B descs covering 2 table rows (paired cols)

    @property
    def npc(self):
        return self.n // self.ncores

    @property
    def nwin(self):
        return (self.npc + P - 1) // P

    @property
    def chunk(self):
        return self.n // self.nchunk

    @property
    def twidth(self):
        return (2 if self.dual else 1) * self.fout


CFG = Config()


def _split_hi_lo(x, dt):
    hi = x.astype(dt)
    lo = (x - hi.astype(np.float32)).astype(dt)
    return hi, lo


# --------------------------------------------------------------------------
# Static edge layout (shared across cores -> one SPMD program)
# --------------------------------------------------------------------------

@dataclass
class Layout:
    cfg: Config
    swc: np.ndarray          # [nwin*nchunk] padded group sizes (mult of 128)
    off: np.ndarray          # [nwin*nchunk] slot offset of each group
    runs: list               # [(g, c, start, length)] gather runs
    groups: list             # list of window-index lists
    total_slots: int = 0
    m16: np.ndarray = None   # static per-(w,c) valid count incl. zero-fill
    notrim: np.ndarray = None  # first-rotation groups: fully gathered

    @property
    def nblocks(self):
        return self.total_slots // P


def build_layout(cfg: Config, row, col):
    """row/col: int arrays [E]. Returns (layout, per-core stream builders)."""
    npc, nwin, nchunk, chunk = cfg.npc, cfg.nwin, cfg.nchunk, cfg.chunk
    ngrp = nwin * nchunk

    per_core = []
    counts = np.zeros((cfg.ncores, ngrp), np.int64)
    for k in range(cfg.ncores):
        sel = (row >= k * npc) & (row < (k + 1) * npc)
        r = (row[sel] - k * npc).astype(np.int64)
        c_ = col[sel].astype(np.int64)
        w = r // P
        rl = r - w * P
        ch = c_ // chunk
        cl = c_ - ch * chunk
        key = w * nchunk + ch
        counts[k] = np.bincount(key, minlength=ngrp)
        per_core.append((key, cl, rl))

    cmax = counts.max(axis=0)
    swc = ((cmax + P - 1) // P) * P               # 128-padded static sizes
    m16 = ((cmax + 15) // 16) * 16                # static valid count (reg)

    groups = [list(range(g, min(g + cfg.gwin, nwin)))
              for g in range(0, nwin, cfg.gwin)]

    off = np.zeros(ngrp, np.int64)
    runs = []
    cur = 0
    for gi, g in enumerate(groups):
        for c in range(nchunk):
            start = cur
            for w in g:
                off[w * nchunk + c] = cur
                cur += swc[w * nchunk + c]
            runs.append((gi, c, start, cur - start))
    layout = Layout(cfg, swc, off, runs, groups, int(cur))
    layout.m16 = m16
    notrim = np.zeros(ngrp, bool)
    for g in groups[:4]:
        for w in g:
            notrim[w * nchunk:(w + 1) * nchunk] = True
    layout.notrim = notrim
    return layout, per_core


def build_phase1_t(cfg: Config):
    """h.T = W.T @ x computed directly in transposed orientation.

    matmul(out[fout, nodes], lhsT=W[finP, fout], rhs=xt[finP, nodes]) with
    512-node PSUM tiles. h2T rows are node-major (1KB per partition per DMA
    descriptor) -> ~8x fewer HWDGE descriptors than row-major h2 writes.
    Host transposes h2T -> h2 between launches.
    """
    nc = bacc.Bacc("TRN2", target_bir_lowering=False, debug=False,
                   num_devices=cfg.ncores)
    assert not cfg.dual
    fin, fout, npc = cfg.fin, cfg.fout, cfg.npc
    nkt = fin // P
    NT = 512                                   # nodes per PSUM tile
    xt = nc.dram_tensor("xt", [fin, npc], BF16, kind="ExternalInput")
    w_hi = nc.dram_tensor("w_hi", [fin, fout], BF16, kind="ExternalInput")
    w_lo = nc.dram_tensor("w_lo", [fin, fout], BF16, kind="ExternalInput")
    h2t = nc.dram_tensor("h2t", [fout, npc], F16, kind="ExternalOutput")

    NXC = 8
    xcb = [(npc * j // NXC // NT * NT) for j in range(NXC)] + [npc]

    with tile.TileContext(nc) as tc, ExitStack() as ctx:
        wpool = ctx.enter_context(tc.tile_pool(name="w", bufs=1))
        xpool = ctx.enter_context(tc.tile_pool(name="x", bufs=1))
        ppool = ctx.enter_context(tc.tile_pool(name="ps", bufs=6, space="PSUM"))
        opool = ctx.enter_context(tc.tile_pool(name="o", bufs=6))

        whs, wls = [], []
        for k in range(nkt):
            wh = wpool.tile([P, fout], BF16, tag=f"wh{k}")
            wl = wpool.tile([P, fout], BF16, tag=f"wl{k}")
            nc.sync.dma_start(wh[:], w_hi.ap()[k * P:(k + 1) * P, :])
            nc.sync.dma_start(wl[:], w_lo.ap()[k * P:(k + 1) * P, :])
            whs.append(wh)
            wls.append(wl)

        xh = [[None] * NXC for _ in range(nkt)]
        for j in range(NXC):
            c0, c1 = xcb[j], xcb[j + 1]
            for k in range(nkt):
                a = xpool.tile([P, c1 - c0], BF16, tag=f"x{k}_{j}")
                nc.sync.dma_start(a[:], xt.ap()[k * P:(k + 1) * P, c0:c1])
                xh[k][j] = a

        for t in range((npc + NT - 1) // NT):
            n0 = t * NT
            nt = min(npc - n0, NT)
            j = next(i for i in range(NXC) if xcb[i] <= n0 < xcb[i + 1])
            o0 = n0 - xcb[j]
            ps = ppool.tile([P, NT], F32)
            mms = []
            for k in range(nkt):
                mms += [(whs[k], xh[k][j]), (wls[k], xh[k][j])]
            for i, (lhsT, rhs) in enumerate(mms):
                nc.tensor.matmul(ps[:, :nt], lhsT[:], rhs[:, o0:o0 + nt],
                                 start=(i == 0), stop=(i == len(mms) - 1))
            ot = opool.tile([P, NT], F16, tag="ot")
            nc.vector.tensor_copy(ot[:, :nt], ps[:, :nt])
            nc.sync.dma_start(h2t.ap()[:, n0:n0 + nt], ot[:, :nt])
    nc.compile()
    return nc


# --------------------------------------------------------------------------
# Paired layout: per-core table permutation pairs cols used together in one
# (w, c) bucket at adjacent table rows; one 512B descriptor fetches both.
# --------------------------------------------------------------------------

@dataclass
class PLayout:
    cfg: Config
    pc: np.ndarray           # [nwin*nchunk] static paired-cell counts
    uc: np.ndarray           # [nwin*nchunk] static single counts
    offp: np.ndarray         # cell offset of bucket within its paired run
    offu: np.ndarray         # slot offset of bucket within its single run
    runs: list               # [(gi, c, kind, s16, L, blk0)] kind: 'p'|'s'
    groups: list
    idx_cols: int            # total idx stream columns (16-wraps)
    nblocks: int             # total ldid blocks
    blockmap: list           # [(tag, c, lb, gb, wi, w, half)]
    ndesc: int = 0


def _pair_core_edges(cfg, key, cl, rl, order_runs):
    """Greedy within-bucket pairing with a per-chunk global partner map.

    Returns per-bucket lists: pairs[(w,c)] = [(i1, i2)], singles[(w,c)] = [i]
    (indices into the core's edge arrays), and per-chunk partner arrays.
    """
    nchunk, chunk = cfg.nchunk, cfg.chunk
    by_bucket = {}
    o = np.argsort(key, kind="stable")
    sk = key[o]
    bounds = np.searchsorted(sk, np.arange(cfg.nwin * nchunk + 1))
    partner = [np.full(chunk, -1, np.int64) for _ in range(nchunk)]
    pairs = {}
    singles = {}
    for (w, c) in order_runs:
        b = w * nchunk + c
        inst = o[bounds[b]:bounds[b + 1]]
        cols = cl[inst]
        par = partner[c]
        used = np.zeros(len(inst), bool)
        ps = []
        # instances by col (multiplicity handling)
        bycol = {}
        for j, cc in enumerate(cols):
            bycol.setdefault(int(cc), []).append(j)
        # 1) cols whose partner is also present
        for cc, js in list(bycol.items()):
            p = par[cc]
            if p >= 0 and p in bycol and cc < p:
                a_js = [j for j in js if not used[j]]
                b_js = [j for j in bycol[p] if not used[j]]
                for ja, jb in zip(a_js, b_js):
                    ps.append((inst[ja], inst[jb]))  # cc < p: even slot first
                    used[ja] = used[jb] = True
        # 2) unmatched cols with free instances -> new partners
        free = [(cc, j) for cc, js in bycol.items() if par[cc] < 0
                for j in js if not used[j]]
        # only one instance per col per round to keep partner bijective
        seen = set()
        free1 = []
        for cc, j in free:
            if cc not in seen:
                seen.add(cc)
                free1.append((cc, j))
        for t in range(0, len(free1) - 1, 2):
            (ca, ja), (cb, jb) = free1[t], free1[t + 1]
            if ca > cb:
                (ca, ja), (cb, jb) = (cb, jb), (ca, ja)
            par[ca] = cb
            par[cb] = ca
            ps.append((inst[ja], inst[jb]))  # smaller col -> even position
            used[ja] = used[jb] = True
        pairs[(w, c)] = ps
        singles[(w, c)] = [inst[j] for j in range(len(inst)) if not used[j]]
    return pairs, singles, partner


def build_layout_paired(cfg: Config, row, col):
    npc, nwin, nchunk, chunk = cfg.npc, cfg.nwin, cfg.nchunk, cfg.chunk
    ngrp = nwin * nchunk
    groups = [list(range(g, min(g + cfg.gwin, nwin)))
              for g in range(0, nwin, cfg.gwin)]
    order_runs = [(w, c) for g in groups for c in range(nchunk) for w in g]

    per_core = []
    npairs = np.zeros((cfg.ncores, ngrp), np.int64)
    nsing = np.zeros((cfg.ncores, ngrp), np.int64)
    for k in range(cfg.ncores):
        sel = (row >= k * npc) & (row < (k + 1) * npc)
        r = (row[sel] - k * npc).astype(np.int64)
        c_ = col[sel].astype(np.int64)
        w = r // P
        rl = r - w * P
        ch = c_ // chunk
        cl = c_ - ch * chunk
        key = w * nchunk + ch
        pairs, singles, partner = _pair_core_edges(cfg, key, cl, rl,
                                                   order_runs)
        for (w_, c2), ps in pairs.items():
            npairs[k, w_ * nchunk + c2] = len(ps)
        for (w_, c2), ss in singles.items():
            nsing[k, w_ * nchunk + c2] = len(ss)
        per_core.append((cl, rl, pairs, singles, partner))

    pc = npairs.max(axis=0)
    uc = nsing.max(axis=0)

    offp = np.zeros(ngrp, np.int64)
    offu = np.zeros(ngrp, np.int64)
    runs = []
    blockmap = []
    s16 = 0      # idx stream cursor (16-granular)
    blk = 0      # ldid block cursor
    ndesc = 0
    for gi, g in enumerate(groups):
        w0 = g[0]
        # paired runs: one per chunk-PAIR c2 (pair idx of 2 chunks fits int16)
        for c2 in range(nchunk // 2):
            cur = 0
            for w in g:
                for c4 in (2 * c2, 2 * c2 + 1):
                    offp[w * nchunk + c4] = cur
                    cur += pc[w * nchunk + c4]
            L = int(cur)
            runs.append((gi, c2, "p", s16, L, blk))
            nb = (L + P - 1) // P
            for lb in range(nb):
                b0, b1 = lb * P, (lb + 1) * P
                for w in g:
                    for c4 in (2 * c2, 2 * c2 + 1):
                        o0 = offp[w * nchunk + c4]
                        o1 = o0 + pc[w * nchunk + c4]
                        if o0 < b1 and o1 > b0:
                            for h in range(2):
                                blockmap.append(
                                    ("p", c2, lb, blk + 2 * lb + h,
                                     w - w0, w, h, gi))
            s16 += ((L + 15) // 16) * 16
            blk += 2 * nb
            ndesc += L
        for c in range(nchunk):
            cur = 0
            for w in g:
                offu[w * nchunk + c] = cur
                cur += uc[w * nchunk + c]
            L = int(cur)
            runs.append((gi, c, "s", s16, L, blk))
            nb = (L + P - 1) // P
            for lb in range(nb):
                b0, b1 = lb * P, (lb + 1) * P
                for w in g:
                    o0 = offu[w * nchunk + c]
                    o1 = o0 + uc[w * nchunk + c]
                    if o0 < b1 and o1 > b0:
                        blockmap.append(
                            ("s", c, lb, blk + lb, w - w0, w, 0, gi))
            s16 += ((L + 15) // 16) * 16
            blk += nb
            ndesc += L
    lay = PLayout(cfg, pc, uc, offp, offu, runs, groups,
                  s16 // 16, blk, blockmap, ndesc)
    return lay, per_core


def build_streams_paired(layout: PLayout, cl, rl, pairs, singles, partner):
    """Per-core idx stream + ldid stream + per-chunk table permutation."""
    cfg = layout.cfg
    nchunk, chunk, gwin = cfg.nchunk, cfg.chunk, cfg.gwin
    idx_flat = np.zeros(layout.idx_cols * 16, np.int16)
    ldid = np.full(layout.nblocks * P, -1.0, np.float32)

    # per-chunk positions: paired cols at 2m, 2m+1; rest anywhere
    pos = [np.full(chunk, -1, np.int64) for _ in range(nchunk)]
    perm = []  # pos -> original col, per chunk
    for c in range(nchunk):
        par = partner[c]
        a = np.where((par >= 0) & (np.arange(chunk) < par))[0]
        p2c = np.empty(chunk, np.int64)
        m = len(a)
        p2c[0:2 * m:2] = a
        p2c[1:2 * m + 1:2] = par[a]
        rest = np.where(par < 0)[0]
        p2c[2 * m:] = rest
        perm.append(p2c)
        pos[c][p2c] = np.arange(chunk)

    run_idx = {(gi, c, kind): (s16, L, blk)
               for (gi, c, kind, s16, L, blk) in layout.runs}
    for gi, g in enumerate(layout.groups):
        w0 = g[0]
        for c in range(nchunk):
            po = pos[c]
            s16p, Lp, blkp = run_idx[(gi, c, "p")]
            s16s, Ls, blks = run_idx[(gi, c, "s")]
            for w in g:
                b = w * nchunk + c
                # paired cells
                o = layout.offp[b]
                for t, (i1, i2) in enumerate(pairs.get((w, c), [])):
                    cell = o + t
                    p1 = po[cl[i1]]
                    assert po[cl[i2]] == p1 + 1 and p1 % 2 == 0
                    idx_flat[(s16p) + cell] = p1 // 2
                    gb = blkp + 2 * (cell // P)
                    pp = cell % P
                    ldid[(gb + 0) * P + pp] = (w - w0) * P + rl[i1]
                    ldid[(gb + 1) * P + pp] = (w - w0) * P + rl[i2]
                # singles
                o = layout.offu[b]
                for t, i1 in enumerate(singles.get((w, c), [])):
                    slot = o + t
                    idx_flat[s16s + slot] = po[cl[i1]]
                    gb = blks + (slot // P)
                    ldid[gb * P + (slot % P)] = (w - w0) * P + rl[i1]

    idx_w = np.zeros((P, layout.idx_cols), np.int16)
    for (gi, c, kind, s16, L, blk) in layout.runs:
        if L == 0:
            continue
        L16 = ((L + 15) // 16) * 16
        seg = idx_flat[s16:s16 + L16].reshape(-1, 16).T
        idx_w[:, s16 // 16:(s16 + L16) // 16] = np.tile(seg, (8, 1))
    ld_w = np.ascontiguousarray(
        ldid.reshape(-1, P).T.astype(np.float16))
    return idx_w, ld_w, perm


def build_phase2_paired(cfg: Config, layout: PLayout):
    nc = bacc.Bacc("TRN2", target_bir_lowering=False, debug=False,
                   num_devices=cfg.ncores, num_swdge_queues=4)
    fout, npc, nchunk, chunk = cfg.fout, cfg.npc, cfg.nchunk, cfg.chunk
    assert not cfg.dual
    tw = cfg.twidth
    SBATCH = 8
    h2 = nc.dram_tensor("h2", [cfg.n, tw], F16, kind="ExternalInput")
    idxs = nc.dram_tensor("idxs", [P, layout.idx_cols], I16,
                          kind="ExternalInput")
    rowloc = nc.dram_tensor("rowloc", [P, layout.nblocks], F16,
                            kind="ExternalInput")
    iota_in = nc.dram_tensor("iota", [P, cfg.gwin * SBATCH * P], F16,
                             kind="ExternalInput")
    out = nc.dram_tensor("out", [npc, fout], F32, kind="ExternalOutput")

    h2p = h2.ap().rearrange("(a b) d -> a (b d)", b=2)  # [n/2, 2tw]

    run_by = {(gi, c, kind): (s16, L, blk)
              for (gi, c, kind, s16, L, blk) in layout.runs}
    bpmax = {c: max((run_by[(gi, c, "p")][1] + P - 1) // P
                    for gi in range(len(layout.groups)))
             for c in range(nchunk)}
    bsmax = {c: max((run_by[(gi, c, "s")][1] + P - 1) // P
                    for gi in range(len(layout.groups)))
             for c in range(nchunk)}
    bm_by_gw = {}
    for (kind, c, lb, gb, wi, w, h, gi) in layout.blockmap:
        bm_by_gw.setdefault((gi, w), []).append((kind, c, lb, gb, wi, h))

    with tile.TileContext(nc) as tc, ExitStack() as ctx:
        nc.gpsimd.load_library(library_config.mlp)

        cpool = ctx.enter_context(tc.tile_pool(name="const", bufs=1))
        mpool = ctx.enter_context(tc.tile_pool(name="msg", bufs=cfg.mbufs))
        spool = ctx.enter_context(tc.tile_pool(name="sel", bufs=6))
        ppool = ctx.enter_context(tc.tile_pool(name="ps", bufs=4, space="PSUM"))
        epool = ctx.enter_context(tc.tile_pool(name="elu", bufs=3))

        iota_t = cpool.tile([P, cfg.gwin * SBATCH * P], F16)
        nc.sync.dma_start(iota_t[:], iota_in.ap()[:, :])
        ld_t = cpool.tile([P, layout.nblocks], F16)
        nc.sync.dma_start(ld_t[:], rowloc.ap()[:, :])
        idx_t = cpool.tile([P, layout.idx_cols], I16)
        nc.sync.dma_start(idx_t[:], idxs.ap()[:, :])

        qrr = 0
        for gi, g in enumerate(layout.groups):
            mts = {}
            for c in range(nchunk):
                for kind in ("p", "s"):
                    s16, L, blk = run_by[(gi, c, kind)]
                    if L == 0:
                        continue
                    ew = 2 * tw if kind == "p" else tw
                    bm = bpmax[c] if kind == "p" else bsmax[c]
                    mt = mpool.tile([P, bm, ew], F16, tag=f"m{kind}{c}")
                    if kind == "p":
                        src = h2p[c * chunk // 2:(c + 1) * chunk // 2, :]
                    else:
                        src = h2.ap()[c * chunk:(c + 1) * chunk, :]
                    nsub = (L + cfg.gmax - 1) // cfg.gmax
                    nbk = (L + P - 1) // P
                    sub = (-(-nbk // nsub)) * P if nsub > 1 else L
                    for j in range(nsub):
                        a = j * sub
                        b = min(L, a + sub)
                        nc.gpsimd.dma_gather(
                            mt[:, a // P:(b + P - 1) // P, :], src,
                            idx_t[:, s16 // 16 + a // 16:
                                  s16 // 16 + (b + 15) // 16],
                            b - a, b - a, ew,
                            single_packet=(b - a <= 1024), queue_num=qrr % 4)
                        qrr += 1
                    mts[(kind, c)] = mt
            for w in g:
                nt = min(npc - w * P, P)
                ot = epool.tile([P, fout], F32, tag="out")
                blocks = bm_by_gw.get((gi, w), [])
                if not blocks:
                    nc.vector.memset(ot[:], 0.0)
                    nc.sync.dma_start(out.ap()[w * P:w * P + nt, :],
                                      ot[:nt, :])
                    continue
                blocks = sorted(blocks, key=lambda t: t[3])
                ps = ppool.tile([P, fout], F32)
                nmm = len(blocks)
                mi = 0
                bi = 0
                while bi < len(blocks):
                    nb = 1
                    while (nb < SBATCH and bi + nb < len(blocks)
                           and blocks[bi + nb][3] == blocks[bi][3] + nb):
                        nb += 1
                    gb0 = blocks[bi][3]
                    wi = blocks[bi][4]
                    io0 = wi * SBATCH * P
                    sel = spool.tile([P, SBATCH * P], F16, tag="sel")
                    nc.vector.tensor_tensor(
                        sel[:, :nb * P], iota_t[:, io0:io0 + nb * P],
                        ld_t[:, gb0:gb0 + nb].to_broadcast([P, nb, P]),
                        op=mybir.AluOpType.is_equal)
                    for j in range(nb):
                        kind, c, lb, _, _, h = blocks[bi + j]
                        st = sel[:, j * P:(j + 1) * P]
                        rhs = mts[(kind, c)][:, lb,
                                             h * fout:(h + 1) * fout]
                        nc.tensor.matmul(ps[:], st, rhs,
                                         start=(mi == 0),
                                         stop=(mi == nmm - 1))
                        mi += 1
                    bi += nb
                tmin = epool.tile([P, fout], F32, tag="tmin")
                texp = epool.tile([P, fout], F32, tag="texp")
                trel = epool.tile([P, fout], F32, tag="trel")
                nc.scalar.activation(tmin[:], ps[:],
                                     mybir.ActivationFunctionType.Relu,
                                     scale=-1.0)
                nc.scalar.activation(texp[:], tmin[:],
                                     mybir.ActivationFunctionType.Exp,
                                     scale=-1.0)
                nc.vector.tensor_scalar(trel[:], ps[:], 0.0, -1.0,
                                        mybir.AluOpType.max,
                                        mybir.AluOpType.add)
                nc.vector.tensor_add(ot[:], texp[:], trel[:])
                nc.sync.dma_start(out.ap()[w * P:w * P + nt, :], ot[:nt, :])
    nc.compile()
    return nc


# --------------------------------------------------------------------------
# Compacted layout: 16-granular (w, c) buckets, straddle blocks, ldid sel
# --------------------------------------------------------------------------

@dataclass
class CLayout:
    cfg: Config
    bsz: np.ndarray          # [nwin*nchunk] 16-granular static bucket sizes
    off: np.ndarray          # [nwin*nchunk] slot offset of each bucket
    runs: list               # [(gi, c, start, length)]
    groups: list             # list of window-index lists
    total_slots: int
    blockmap: list           # [(c, lb, gb, wi, psum_w)] per sel+matmul

    @property
    def nblocks(self):
        return self.total_slots // P


def build_layout_compact(cfg: Config, row, col):
    npc, nwin, nchunk, chunk = cfg.npc, cfg.nwin, cfg.nchunk, cfg.chunk
    ngrp = nwin * nchunk

    per_core = []
    counts = np.zeros((cfg.ncores, ngrp), np.int64)
    for k in range(cfg.ncores):
        sel = (row >= k * npc) & (row < (k + 1) * npc)
        r = (row[sel] - k * npc).astype(np.int64)
        c_ = col[sel].astype(np.int64)
        w = r // P
        rl = r - w * P
        ch = c_ // chunk
        cl = c_ - ch * chunk
        key = w * nchunk + ch
        counts[k] = np.bincount(key, minlength=ngrp)
        per_core.append((key, cl, rl))

    cmax = counts.max(axis=0)
    bsz = ((cmax + 15) // 16) * 16

    groups = [list(range(g, min(g + cfg.gwin, nwin)))
              for g in range(0, nwin, cfg.gwin)]

    off = np.zeros(ngrp, np.int64)
    runs = []
    blockmap = []
    cur = 0
    for gi, g in enumerate(groups):
        w0 = g[0]
        for c in range(cfg.nchunk):
            start = cur
            for w in g:
                off[w * nchunk + c] = cur
                cur += bsz[w * nchunk + c]
            L = cur - start
            Lp = ((L + P - 1) // P) * P
            cur = start + Lp
            runs.append((gi, c, start, Lp))
            # block -> overlapping windows
            for lb in range(Lp // P):
                b0, b1 = start + lb * P, start + (lb + 1) * P
                for w in g:
                    o0 = off[w * nchunk + c]
                    o1 = o0 + bsz[w * nchunk + c]
                    if o0 < b1 and o1 > b0:
                        blockmap.append((c, lb, (b0 // P), w - w0, w))
    return CLayout(cfg, bsz, off, runs, groups, int(cur), blockmap), per_core


def build_streams_compact(layout: CLayout, key, cl, rl):
    """idx stream (0-padded) + ldid stream (-1-padded) per core."""
    cfg = layout.cfg
    nchunk = cfg.nchunk
    total = layout.total_slots
    idx_local = np.zeros(total, np.int16)
    ldid = np.full(total, -1.0, np.float32)

    order = np.argsort(key, kind="stable")
    skey = key[order]
    scl = cl[order]
    srl = rl[order]
    cnt = np.bincount(key, minlength=len(layout.bsz))
    starts = np.concatenate([[0], np.cumsum(cnt)[:-1]])
    rank = np.arange(len(skey)) - starts[skey]
    slot = layout.off[skey] + rank
    idx_local[slot] = scl.astype(np.int16)
    w_of_key = skey // nchunk
    w0_of_key = np.array(
        [layout.groups[wi // cfg.gwin][0] for wi in range(cfg.nwin)],
        np.int64)[w_of_key]
    ldid[slot] = (w_of_key - w0_of_key) * P + srl

    idx_w = np.zeros((P, total // 16), np.int16)
    for (_, _, s, L) in layout.runs:
        if L == 0:
            continue
        seg = idx_local[s:s + L].reshape(-1, 16).T
        idx_w[:, s // 16:(s + L) // 16] = np.tile(seg, (8, 1))
    ld_w = np.ascontiguousarray(ldid.reshape(-1, P).T.astype(np.float16))
    return idx_w, ld_w


def build_phase2_compact(cfg: Config, layout: CLayout):
    nc = bacc.Bacc("TRN2", target_bir_lowering=False, debug=False,
                   num_devices=cfg.ncores, num_swdge_queues=4)
    fout, npc, nchunk, chunk = cfg.fout, cfg.npc, cfg.nchunk, cfg.chunk
    tw = cfg.twidth
    SBATCH = 8
    h2 = nc.dram_tensor("h2", [cfg.n, tw], F16, kind="ExternalInput")
    idxs = nc.dram_tensor("idxs", [P, layout.total_slots // 16], I16,
                          kind="ExternalInput")
    rowloc = nc.dram_tensor("rowloc", [P, layout.nblocks], F16,
                            kind="ExternalInput")
    iota_in = nc.dram_tensor("iota", [P, cfg.gwin * SBATCH * P], F16,
                             kind="ExternalInput")
    out = nc.dram_tensor("out", [npc, fout], F32, kind="ExternalOutput")

    run_by_gc = {(gi, c): (s, L) for (gi, c, s, L) in layout.runs}
    bmax = {c: max(run_by_gc[(gi, c)][1] // P
                   for gi in range(len(layout.groups))) for c in range(nchunk)}
    bm_by_gw = {}
    for (c, lb, gb, wi, w) in layout.blockmap:
        gi = next(i for i, g in enumerate(layout.groups) if w in g)
        bm_by_gw.setdefault((gi, w), []).append((c, lb, gb, wi))

    with tile.TileContext(nc) as tc, ExitStack() as ctx:
        nc.gpsimd.load_library(library_config.mlp)

        cpool = ctx.enter_context(tc.tile_pool(name="const", bufs=1))
        mpool = ctx.enter_context(tc.tile_pool(name="msg", bufs=cfg.mbufs))
        spool = ctx.enter_context(tc.tile_pool(name="sel", bufs=6))
        ppool = ctx.enter_context(tc.tile_pool(name="ps", bufs=4, space="PSUM"))
        epool = ctx.enter_context(tc.tile_pool(name="elu", bufs=3))

        iota_t = cpool.tile([P, cfg.gwin * SBATCH * P], F16)
        nc.sync.dma_start(iota_t[:], iota_in.ap()[:, :])
        ld_t = cpool.tile([P, layout.nblocks], F16)
        nc.sync.dma_start(ld_t[:], rowloc.ap()[:, :])
        idx_t = cpool.tile([P, layout.total_slots // 16], I16)
        nc.sync.dma_start(idx_t[:], idxs.ap()[:, :])

        qrr = 0
        for gi, g in enumerate(layout.groups):
            mts = {}
            for c in range(nchunk):
                s, L = run_by_gc[(gi, c)]
                if L == 0:
                    continue
                mt = mpool.tile([P, bmax[c], tw], F16, tag=f"msg{c}")
                nsub = (L + cfg.gmax - 1) // cfg.gmax
                sub = -(-(L // P) // nsub) * P
                for o in range(s, s + L, sub):
                    b = min(o + sub, s + L)
                    nc.gpsimd.dma_gather(
                        mt[:, (o - s) // P:(b - s) // P, :],
                        h2.ap()[c * chunk:(c + 1) * chunk, :],
                        idx_t[:, o // 16:b // 16], b - o, b - o, tw,
                        single_packet=(b - o <= 1024), queue_num=qrr % 4)
                    qrr += 1
                mts[c] = mt
            for w in g:
                nt = min(npc - w * P, P)
                ot = epool.tile([P, fout], F32, tag="out")
                blocks = bm_by_gw.get((gi, w), [])
                if not blocks:
                    nc.vector.memset(ot[:], 0.0)
                    nc.sync.dma_start(out.ap()[w * P:w * P + nt, :], ot[:nt, :])
                    continue
                ps = ppool.tile([P, fout], F32)
                nmm = len(blocks) * (2 if cfg.dual else 1)
                mi = 0
                # batched one-hot builds over consecutive global blocks; for a
                # given window all entries share the same wi (iota base)
                bi = 0
                while bi < len(blocks):
                    nb = 1
                    while (nb < SBATCH and bi + nb < len(blocks)
                           and blocks[bi + nb][2] == blocks[bi][2] + nb):
                        nb += 1
                    gb0 = blocks[bi][2]
                    wi = blocks[bi][3]
                    io0 = wi * SBATCH * P
                    sel = spool.tile([P, SBATCH * P], F16, tag="sel")
                    nc.vector.tensor_tensor(
                        sel[:, :nb * P], iota_t[:, io0:io0 + nb * P],
                        ld_t[:, gb0:gb0 + nb].to_broadcast([P, nb, P]),
                        op=mybir.AluOpType.is_equal)
                    for j in range(nb):
                        c, lb, _, _ = blocks[bi + j]
                        st = sel[:, j * P:(j + 1) * P]
                        nc.tensor.matmul(ps[:], st, mts[c][:, lb, 0:fout],
                                         start=(mi == 0), stop=(mi == nmm - 1))
                        mi += 1
                        if cfg.dual:
                            nc.tensor.matmul(ps[:], st,
                                             mts[c][:, lb, fout:2 * fout],
                                             start=False, stop=(mi == nmm - 1))
                            mi += 1
                    bi += nb
                tmin = epool.tile([P, fout], F32, tag="tmin")
                texp = epool.tile([P, fout], F32, tag="texp")
                trel = epool.tile([P, fout], F32, tag="trel")
                nc.scalar.activation(tmin[:], ps[:],
                                     mybir.ActivationFunctionType.Relu,
                                     scale=-1.0)
                nc.scalar.activation(texp[:], tmin[:],
                                     mybir.ActivationFunctionType.Exp,
                                     scale=-1.0)
                nc.vector.tensor_scalar(trel[:], ps[:], 0.0, -1.0,
                                        mybir.AluOpType.max,
                                        mybir.AluOpType.add)
                nc.vector.tensor_add(ot[:], texp[:], trel[:])
                nc.sync.dma_start(out.ap()[w * P:w * P + nt, :], ot[:nt, :])
    nc.compile()
    return nc


def build_streams(layout: Layout, key, cl, rl):
    """Per-core edge streams: wrapped int16 gather indices + row-local values."""
    cfg = layout.cfg
    total = layout.total_slots
    pad = -1 if cfg.trim else 0
    idx_local = np.full(total, pad, np.int16)
    rowloc = np.full(total, -1.0, np.float32)

    order = np.argsort(key, kind="stable")
    skey = key[order]
    scl = cl[order]
    srl = rl[order]
    cnt = np.bincount(key, minlength=len(layout.swc))
    starts = np.concatenate([[0], np.cumsum(cnt)[:-1]])
    rank = np.arange(len(skey)) - starts[skey]
    slot = layout.off[skey] + rank
    idx_local[slot] = scl.astype(np.int16)
    rowloc[slot] = srl

    if cfg.trim:
        for gidx in range(len(layout.swc)):
            o = int(layout.off[gidx])
            if layout.notrim[gidx]:
                b = int(layout.swc[gidx])
            else:
                b = int(layout.m16[gidx])
            a = int(cnt[gidx])
            if b > a:
                idx_local[o + a:o + b] = 0
    idx_w = np.zeros((P, total // 16), np.int16)
    for (_, _, s, L) in layout.runs:
        if L == 0:
            continue
        seg = idx_local[s:s + L].reshape(-1, 16).T          # [16, L/16]
        idx_w[:, s // 16:(s + L) // 16] = np.tile(seg, (8, 1))
    rdt = np.float32 if cfg.sptr else np.float16
    rl_w = np.ascontiguousarray(
        rowloc.reshape(-1, P).T.astype(rdt))                 # [128, nblocks]
    return idx_w, rl_w


# --------------------------------------------------------------------------
# Launch 1: h = x @ W  (node-sharded), emit interleaved fp16 table
# --------------------------------------------------------------------------

def build_phase1(cfg: Config):
    nc = bacc.Bacc("TRN2", target_bir_lowering=False, debug=False,
                   num_devices=cfg.ncores)
    fin, fout, npc = cfg.fin, cfg.fout, cfg.npc
    nkt = fin // P
    xt_hi = nc.dram_tensor("xt_hi", [fin, npc], BF16, kind="ExternalInput")
    xt_lo = nc.dram_tensor("xt_lo", [fin, npc], BF16, kind="ExternalInput")
    w_hi = nc.dram_tensor("w_hi", [fin, fout], BF16, kind="ExternalInput")
    w_lo = nc.dram_tensor("w_lo", [fin, fout], BF16, kind="ExternalInput")
    h2 = nc.dram_tensor("h2", [npc, cfg.twidth], F16, kind="ExternalOutput")

    with tile.TileContext(nc) as tc, ExitStack() as ctx:
        wpool = ctx.enter_context(tc.tile_pool(name="w", bufs=1))
        xpool = ctx.enter_context(tc.tile_pool(name="x", bufs=1))
        ppool = ctx.enter_context(tc.tile_pool(name="ps", bufs=4, space="PSUM"))
        opool = ctx.enter_context(tc.tile_pool(name="o", bufs=4))

        whs, wls = [], []
        for k in range(nkt):
            wh = wpool.tile([P, fout], BF16, tag=f"wh{k}")
            wl = wpool.tile([P, fout], BF16, tag=f"wl{k}")
            nc.sync.dma_start(wh[:], w_hi.ap()[k * P:(k + 1) * P, :])
            nc.sync.dma_start(wl[:], w_lo.ap()[k * P:(k + 1) * P, :])
            whs.append(wh)
            wls.append(wl)

        # Whole x-transpose slice resident in SBUF: 4 big line-rate DMAs
        # instead of 4 small ones per node tile (HWDGE issue overhead).
        xh, xl = [], []
        for k in range(nkt):
            a = xpool.tile([P, npc], BF16, tag=f"xh{k}")
            b = xpool.tile([P, npc], BF16, tag=f"xl{k}")
            nc.sync.dma_start(a[:], xt_hi.ap()[k * P:(k + 1) * P, :])
            nc.sync.dma_start(b[:], xt_lo.ap()[k * P:(k + 1) * P, :])
            xh.append(a)
            xl.append(b)

        for t in range(cfg.nwin):
            n0 = t * P
            nt = min(npc - n0, P)
            ps = ppool.tile([P, fout], F32)
            mms = []
            for k in range(nkt):
                mms += [(xh[k], whs[k]), (xh[k], wls[k]), (xl[k], whs[k])]
            for j, (lhsT, rhs) in enumerate(mms):
                nc.tensor.matmul(ps[:nt, :], lhsT[:, n0:n0 + nt], rhs[:],
                                 start=(j == 0), stop=(j == len(mms) - 1))
            ot = opool.tile([P, cfg.twidth], F16, tag="ot")
            nc.vector.tensor_copy(ot[:nt, 0:fout], ps[:nt, :])
            if cfg.dual:
                t32 = opool.tile([P, fout], F32, tag="t32")
                nc.vector.tensor_copy(t32[:nt, :], ot[:nt, 0:fout])
                nc.vector.tensor_tensor(ot[:nt, fout:2 * fout], ps[:nt, :],
                                        t32[:nt, :], op=mybir.AluOpType.subtract)
            nc.sync.dma_start(h2.ap()[n0:n0 + nt, :], ot[:nt, :])
    nc.compile()
    return nc


def build_phase1_single(cfg: Config):
    """h = x @ W with x in single bf16 (W dual-bf16): 2 matmuls per k-tile.

    x loads are chunked along the node axis so the first window's matmuls
    start after ~1/8 of the x slice has landed instead of all of it.
    """
    nc = bacc.Bacc("TRN2", target_bir_lowering=False, debug=False,
                   num_devices=cfg.ncores)
    fin, fout, npc = cfg.fin, cfg.fout, cfg.npc
    nkt = fin // P
    xt = nc.dram_tensor("xt", [fin, npc], BF16, kind="ExternalInput")
    w_hi = nc.dram_tensor("w_hi", [fin, fout], BF16, kind="ExternalInput")
    w_lo = nc.dram_tensor("w_lo", [fin, fout], BF16, kind="ExternalInput")
    h2 = nc.dram_tensor("h2", [npc, cfg.twidth], F16, kind="ExternalOutput")

    NXC = 8  # x column chunks
    xcb = [(npc * j // NXC // P * P) for j in range(NXC)] + [npc]

    with tile.TileContext(nc) as tc, ExitStack() as ctx:
        wpool = ctx.enter_context(tc.tile_pool(name="w", bufs=1))
        xpool = ctx.enter_context(tc.tile_pool(name="x", bufs=1))
        ppool = ctx.enter_context(tc.tile_pool(name="ps", bufs=4, space="PSUM"))
        opool = ctx.enter_context(tc.tile_pool(name="o", bufs=4))

        whs, wls = [], []
        for k in range(nkt):
            wh = wpool.tile([P, fout], BF16, tag=f"wh{k}")
            wl = wpool.tile([P, fout], BF16, tag=f"wl{k}")
            nc.sync.dma_start(wh[:], w_hi.ap()[k * P:(k + 1) * P, :])
            nc.sync.dma_start(wl[:], w_lo.ap()[k * P:(k + 1) * P, :])
            whs.append(wh)
            wls.append(wl)

        xh = [[None] * NXC for _ in range(nkt)]
        for j in range(NXC):
            c0, c1 = xcb[j], xcb[j + 1]
            for k in range(nkt):
                a = xpool.tile([P, c1 - c0], BF16, tag=f"xh{k}_{j}")
                nc.sync.dma_start(a[:], xt.ap()[k * P:(k + 1) * P, c0:c1])
                xh[k][j] = a

        for t in range(cfg.nwin):
            n0 = t * P
            nt = min(npc - n0, P)
            j = next(i for i in range(NXC) if xcb[i] <= n0 < xcb[i + 1])
            o0 = n0 - xcb[j]
            ps = ppool.tile([P, fout], F32)
            mms = []
            for k in range(nkt):
                mms += [(xh[k][j], whs[k]), (xh[k][j], wls[k])]
            for i, (lhsT, rhs) in enumerate(mms):
                nc.tensor.matmul(ps[:nt, :], lhsT[:, o0:o0 + nt], rhs[:],
                                 start=(i == 0), stop=(i == len(mms) - 1))
            ot = opool.tile([P, cfg.twidth], F16, tag="ot")
            nc.vector.tensor_copy(ot[:nt, 0:fout], ps[:nt, :])
            if cfg.dual:
                t32 = opool.tile([P, fout], F32, tag="t32")
                nc.vector.tensor_copy(t32[:nt, :], ot[:nt, 0:fout])
                nc.vector.tensor_tensor(ot[:nt, fout:2 * fout], ps[:nt, :],
                                        t32[:nt, :], op=mybir.AluOpType.subtract)
            nc.sync.dma_start(h2.ap()[n0:n0 + nt, :], ot[:nt, :])
    nc.compile()
    return nc


# --------------------------------------------------------------------------
# Launch 2: gather + segment-sum (one-hot matmul) + ELU
# --------------------------------------------------------------------------

def build_phase2(cfg: Config, layout: Layout):
    nc = bacc.Bacc("TRN2", target_bir_lowering=False, debug=False,
                   num_devices=cfg.ncores, num_swdge_queues=4)
    fout, npc, nchunk, chunk = cfg.fout, cfg.npc, cfg.nchunk, cfg.chunk
    tw = cfg.twidth
    h2 = nc.dram_tensor("h2", [cfg.n, tw], F16, kind="ExternalInput")
    idxs = nc.dram_tensor("idxs", [P, layout.total_slots // 16], I16,
                          kind="ExternalInput")
    RLDT = F32 if cfg.sptr else F16  # scalar-ptr operand must be fp32
    rowloc = nc.dram_tensor("rowloc", [P, layout.nblocks], RLDT,
                            kind="ExternalInput")
    SBATCH = 8  # one-hot builds batched over consecutive blocks
    iota_in = nc.dram_tensor("iota", [P, SBATCH * P], F16, kind="ExternalInput")
    out = nc.dram_tensor("out", [npc, fout], F32, kind="ExternalOutput")

    run_by_gc = {(g, c): (s, L) for (g, c, s, L) in layout.runs}
    bmax = {c: max((run_by_gc[(gi, c)][1] // P)
                   for gi in range(len(layout.groups))) for c in range(nchunk)}

    with tile.TileContext(nc) as tc, ExitStack() as ctx:
        nc.gpsimd.load_library(library_config.mlp)

        cpool = ctx.enter_context(tc.tile_pool(name="const", bufs=1))
        MBUFS = cfg.mbufs
        mpool = ctx.enter_context(tc.tile_pool(name="msg", bufs=MBUFS))
        spool = ctx.enter_context(tc.tile_pool(name="sel", bufs=6))
        ppool = ctx.enter_context(tc.tile_pool(name="ps", bufs=4, space="PSUM"))
        epool = ctx.enter_context(tc.tile_pool(name="elu", bufs=3))

        iota_t = cpool.tile([P, SBATCH * P], F16)
        nc.sync.dma_start(iota_t[:], iota_in.ap()[:, :])
        rl_t = cpool.tile([P, layout.nblocks], RLDT)
        nc.sync.dma_start(rl_t[:], rowloc.ap()[:, :])
        # whole idx stream resident: removes per-run DMAs + waits from the
        # gather issue path
        idx_t = cpool.tile([P, layout.total_slots // 16], I16)
        nc.sync.dma_start(idx_t[:], idxs.ap()[:, :])

        qrr = [0]  # round-robin queue counter (adjacent instrs on distinct queues)
        for gi, g in enumerate(layout.groups):
            mts = {}
            for c in range(nchunk):
                s, L = run_by_gc[(gi, c)]
                if L == 0:
                    continue
                mt = mpool.tile([P, bmax[c], tw], F16, tag=f"msg{c}")
                # single_packet=True (fast CounterMachine path) caps one
                # gather at 64 descs/engine = 1024 indices. With trim, one
                # gather per (window, chunk) subgroup so that -1 padding is
                # trailing within each instruction (the Q7 trims it).
                if cfg.trim:
                    bounds = []
                    for w in g:
                        sw = int(layout.swc[w * nchunk + c])
                        o = int(layout.off[w * nchunk + c])
                        mv = int(layout.m16[w * nchunk + c])
                        nsub = (sw + 1023) // 1024
                        full = bool(layout.notrim[w * nchunk + c])
                        for j in range(nsub):
                            sub = -(-(sw // P) // nsub) * P
                            a = o + j * sub
                            b = min(o + sw, a + sub)
                            if b > a:
                                rv = (b - a if full else
                                      min(max(mv - (a - o), 0), b - a))
                                bounds.append((a, b, rv))
                else:
                    nsub = (L + cfg.gmax - 1) // cfg.gmax
                    sub = -(-(L // P) // nsub) * P
                    bounds = [(o, min(o + sub, s + L), min(o + sub, s + L) - o)
                              for o in range(s, s + L, sub)]
                for (a, b, rv) in bounds:
                    Lg = b - a
                    nc.gpsimd.dma_gather(
                        mt[:, (a - s) // P:(b - s) // P, :],
                        h2.ap()[c * chunk:(c + 1) * chunk, :],
                        idx_t[:, a // 16:b // 16], Lg, rv, tw,
                        single_packet=(Lg <= 1024), queue_num=qrr[0] % 4)
                    qrr[0] += 1
                mts[c] = mt
            for w in g:
                blocks = []
                for c in range(nchunk):
                    o = layout.off[w * nchunk + c]
                    s, _ = run_by_gc[(gi, c)]
                    nb = layout.swc[w * nchunk + c] // P
                    for j in range(nb):
                        blocks.append((c, (o - s) // P + j, o // P + j))
                nt = min(npc - w * P, P)
                ot = epool.tile([P, fout], F32, tag="out")
                if not blocks:
                    nc.vector.memset(ot[:], 0.0)
                    nc.sync.dma_start(out.ap()[w * P:w * P + nt, :], ot[:nt, :])
                    continue
                ps = ppool.tile([P, fout], F32)
                nmm = len(blocks) * (2 if cfg.dual else 1)
                mi = 0
                if cfg.sptr:
                    # per-block one-hot via tensor_scalar with per-partition
                    # scalar (InstTensorScalarPtr -> 4x DVE mode)
                    for (c, lb, gb) in blocks:
                        st = spool.tile([P, P], F16, tag="sel")
                        nc.vector.tensor_scalar(
                            st[:], iota_t[:, :P], rl_t[:, gb:gb + 1], None,
                            mybir.AluOpType.is_equal)
                        nc.tensor.matmul(ps[:], st[:], mts[c][:, lb, 0:fout],
                                         start=(mi == 0), stop=(mi == nmm - 1))
                        mi += 1
                        if cfg.dual:
                            nc.tensor.matmul(ps[:], st[:],
                                             mts[c][:, lb, fout:2 * fout],
                                             start=False, stop=(mi == nmm - 1))
                            mi += 1
                    blocks = []
                # batch one-hot builds over runs of consecutive global blocks
                bi = 0
                while bi < len(blocks):
                    nb = 1
                    while (nb < SBATCH and bi + nb < len(blocks)
                           and blocks[bi + nb][2] == blocks[bi][2] + nb):
                        nb += 1
                    gb0 = blocks[bi][2]
                    sel = spool.tile([P, SBATCH * P], F16, tag="sel")
                    nc.vector.tensor_tensor(
                        sel[:, :nb * P], iota_t[:, :nb * P],
                        rl_t[:, gb0:gb0 + nb].to_broadcast([P, nb, P]),
                        op=mybir.AluOpType.is_equal)
                    for j in range(nb):
                        c, lb, _ = blocks[bi + j]
                        st = sel[:, j * P:(j + 1) * P]
                        nc.tensor.matmul(ps[:], st, mts[c][:, lb, 0:fout],
                                         start=(mi == 0), stop=(mi == nmm - 1))
                        mi += 1
                        if cfg.dual:
                            nc.tensor.matmul(ps[:], st,
                                             mts[c][:, lb, fout:2 * fout],
                                             start=False, stop=(mi == nmm - 1))
                            mi += 1
                    bi += nb
                # ELU: relu(x) - 1 + exp(min(x, 0))
                tmin = epool.tile([P, fout], F32, tag="tmin")
                texp = epool.tile([P, fout], F32, tag="texp")
                trel = epool.tile([P, fout], F32, tag="trel")
                nc.scalar.activation(tmin[:], ps[:],
                                     mybir.ActivationFunctionType.Relu,
                                     scale=-1.0)
                nc.scalar.activation(texp[:], tmin[:],
                                     mybir.ActivationFunctionType.Exp,
                                     scale=-1.0)
                nc.vector.tensor_scalar(trel[:], ps[:], 0.0, -1.0,
                                        mybir.AluOpType.max,
                                        mybir.AluOpType.add)
                nc.vector.tensor_add(ot[:], texp[:], trel[:])
                nc.sync.dma_start(out.ap()[w * P:w * P + nt, :], ot[:nt, :])
    nc.compile()
    return nc


# --------------------------------------------------------------------------
# Host orchestration
# --------------------------------------------------------------------------

_P1_CACHE = {}
_P2_CACHE = {}


def _phase1_nc(cfg: Config):
    key = (cfg.n, cfg.fin, cfg.fout, cfg.ncores, cfg.dual, cfg.p1single,
           cfg.p1t)
    if key not in _P1_CACHE:
        _P1_CACHE[key] = (build_phase1_t(cfg) if cfg.p1t
                          else build_phase1_single(cfg) if cfg.p1single
                          else build_phase1(cfg))
    return _P1_CACHE[key]


def _phase2_nc(cfg: Config, layout):
    if cfg.paired:
        sizes = tuple(layout.pc.tolist()) + tuple(layout.uc.tolist())
    elif cfg.compact:
        sizes = tuple(layout.bsz.tolist())
    else:
        sizes = tuple(layout.swc.tolist())
    key = (cfg.n, cfg.fin, cfg.fout, cfg.ncores, cfg.dual, cfg.gwin,
           cfg.trim, cfg.gmax, cfg.mbufs, cfg.sptr, cfg.compact,
           cfg.paired, sizes)
    if key not in _P2_CACHE:
        _P2_CACHE[key] = (build_phase2_paired(cfg, layout) if cfg.paired
                          else build_phase2_compact(cfg, layout)
                          if cfg.compact else build_phase2(cfg, layout))
    return _P2_CACHE[key]


def run(x, edge_index, W, a=None, cfg: Config = CFG, trace=False):
    """Full pipeline; returns (out, info dict with exec times)."""
    x = np.asarray(x, np.float32)
    W = np.asarray(W, np.float32)
    edge_index = np.asarray(edge_index)
    row = edge_index[0].astype(np.int64)
    col = edge_index[1].astype(np.int64)
    npc = cfg.npc
    info = {}

    # ---- phase 1 ----
    x_hi, x_lo = _split_hi_lo(x, ml_dtypes.bfloat16)
    w_hi, w_lo = _split_hi_lo(W, ml_dtypes.bfloat16)
    xt_hi = np.ascontiguousarray(x_hi.T)
    nc1 = _phase1_nc(cfg)
    if cfg.p1single or cfg.p1t:
        in1 = [{
            "xt": np.ascontiguousarray(xt_hi[:, k * npc:(k + 1) * npc]),
            "w_hi": w_hi, "w_lo": w_lo,
        } for k in range(cfg.ncores)]
    else:
        xt_lo = np.ascontiguousarray(x_lo.T)
        in1 = [{
            "xt_hi": np.ascontiguousarray(xt_hi[:, k * npc:(k + 1) * npc]),
            "xt_lo": np.ascontiguousarray(xt_lo[:, k * npc:(k + 1) * npc]),
            "w_hi": w_hi, "w_lo": w_lo,
        } for k in range(cfg.ncores)]
    r1 = run_bass_kernel_spmd(nc1, in1, list(range(cfg.ncores)), trace=trace)
    if cfg.p1t:
        h2 = np.concatenate(
            [np.ascontiguousarray(r1.results[k]["h2t"].T)
             for k in range(cfg.ncores)], axis=0)
    else:
        h2 = np.concatenate([r1.results[k]["h2"] for k in range(cfg.ncores)],
                            axis=0)
    info["p1_ns"] = r1.exec_time_ns

    # ---- layout + streams ----
    if cfg.paired:
        layout, per_core = build_layout_paired(cfg, row, col)
        nc2 = _phase2_nc(cfg, layout)
        SB = 8
        io = np.concatenate([
            np.tile(np.arange(wi * P, (wi + 1) * P, dtype=np.float16), SB)
            for wi in range(cfg.gwin)])
        iota = np.ascontiguousarray(np.broadcast_to(io, (P, cfg.gwin * SB * P)))
        chunk = cfg.chunk
        in2 = []
        for k in range(cfg.ncores):
            cl, rl, pairs, singles, partner = per_core[k]
            idx_w, rl_w, perm = build_streams_paired(layout, cl, rl, pairs,
                                                     singles, partner)
            # per-core permuted table: row (c*chunk + p) = h2[c*chunk+perm[c][p]]
            gperm = np.concatenate(
                [c * chunk + perm[c] for c in range(cfg.nchunk)])
            h2k = np.ascontiguousarray(h2[gperm])
            in2.append({"h2": h2k, "idxs": idx_w, "rowloc": rl_w,
                        "iota": iota})
    elif cfg.compact:
        layout, per_core = build_layout_compact(cfg, row, col)
        nc2 = _phase2_nc(cfg, layout)
        SB = 8
        io = np.concatenate([
            np.tile(np.arange(wi * P, (wi + 1) * P, dtype=np.float16), SB)
            for wi in range(cfg.gwin)])
        iota = np.ascontiguousarray(np.broadcast_to(io, (P, cfg.gwin * SB * P)))
        in2 = []
        for k in range(cfg.ncores):
            idx_w, rl_w = build_streams_compact(layout, *per_core[k])
            in2.append({"h2": h2, "idxs": idx_w, "rowloc": rl_w, "iota": iota})
    else:
        layout, per_core = build_layout(cfg, row, col)
        nc2 = _phase2_nc(cfg, layout)
        iota = np.ascontiguousarray(np.broadcast_to(
            np.tile(np.arange(P, dtype=np.float16), 8), (P, 8 * P)))
        in2 = []
        for k in range(cfg.ncores):
            idx_w, rl_w = build_streams(layout, *per_core[k])
            in2.append({"h2": h2, "idxs": idx_w, "rowloc": rl_w, "iota": iota})
    r2 = run_bass_kernel_spmd(nc2, in2, list(range(cfg.ncores)), trace=trace)
    out = np.concatenate([r2.results[k]["out"] for k in range(cfg.ncores)],
                         axis=0)
    info["p2_ns"] = r2.exec_time_ns
    info["total_slots"] = (layout.ndesc if cfg.paired
                           else layout.total_slots)
    info["results"] = (r1, r2)
    return out, info


def kernel(x, edge_index, W, a=None, **_ignored):
    out, _ = run(x, edge_index, W, a)
    return out

